# revision 1
# baseline (speedup 1.0000x reference)
"""Distributed Trainium2 Bass kernel for a 4-layer GPT-style transformer.

Sharding: 8 cores = 2 batch groups x 4 vocab shards.
  - core c: batch element g = c//4, vocab shard j = c%4 (12672 ids, padded).
  - Transformer body computed per batch element (replicated within each
    group of 4); tied LM head sharded over vocab.  No collectives.

On-chip layout: activations transposed (features on partitions, tokens on
free).  LayerNorm stats via ones-matmul partition reductions; attention via
transposed scores (k @ q^T) so probabilities land keys-on-partitions, ready
for the A@V matmul with no transposes.  Softmax skips max-subtraction
(|scores| < ~2 by construction); causality = 0/1 mask multiply after exp,
only on diagonal-crossing tiles.  Matmuls bf16, residual stream fp32.
Big weight matrices are streamed from DRAM per output tile.
"""

import numpy as np
import ml_dtypes

import concourse.bass as bass
import concourse.mybir as mybir
import concourse.tile as tile
from concourse import bacc
from concourse.bass_utils import run_bass_kernel_spmd

V, E, NH, HD, L, T, B, FF = 50257, 768, 12, 64, 4, 1024, 2, 3072
EPS = 1e-5
P = 128
KE = E // P            # 6 feature subtiles
KF = FF // P           # 24
NT = T // P            # 8 token tiles
NC = 512               # matmul free-dim chunk
NCH = T // NC          # 2 chunks
VP = 12672             # vocab shard per core (99 * 128)
MV = VP // P           # 99
BF16 = mybir.dt.bfloat16
F32 = mybir.dt.float32
AF = mybir.ActivationFunctionType
OP = mybir.AluOpType
BF = ml_dtypes.bfloat16

_CACHE = {}


def _build():
    nc = bacc.Bacc("TRN2", target_bir_lowering=False, debug=False,
                   num_devices=8)

    x0t = nc.declare_dram_parameter("x0t", [E, T], F32, isOutput=False)
    wqk = nc.declare_dram_parameter("wqk", [L, E, 2 * E], BF16, isOutput=False)
    wv = nc.declare_dram_parameter("wv", [L, E, E], BF16, isOutput=False)
    wout = nc.declare_dram_parameter("wout", [L, E, E], BF16, isOutput=False)
    wfc1 = nc.declare_dram_parameter("wfc1", [L, E, FF], BF16, isOutput=False)
    bfc1 = nc.declare_dram_parameter("bfc1", [L, P, KF], F32, isOutput=False)
    wfc2 = nc.declare_dram_parameter("wfc2", [L, FF, E], BF16, isOutput=False)
    bfc2 = nc.declare_dram_parameter("bfc2", [L, P, KE], F32, isOutput=False)
    wemb = nc.declare_dram_parameter("wemb", [E, VP], BF16, isOutput=False)
    maskp = nc.declare_dram_parameter("mask", [4, P, NC], BF16, isOutput=False)
    out = nc.declare_dram_parameter("out", [VP, T], F32, isOutput=True)

    with tile.TileContext(nc) as tc:
        with (
            tc.tile_pool(name="resident", bufs=1) as res,
            tc.tile_pool(name="wts", bufs=1) as wpool,
            tc.tile_pool(name="acts", bufs=1) as apool,
            tc.tile_pool(name="wstream", bufs=3) as wst,
            tc.tile_pool(name="small", bufs=3) as spool,
            tc.tile_pool(name="small2", bufs=2) as spool2,
            tc.tile_pool(name="ps", bufs=2, space="PSUM") as psp,
        ):
            # --- resident tiles ---
            x = res.tile([P, KE, T], F32)          # residual stream (xT)
            xhat = res.tile([P, KE, T], BF16)      # normalized, bf16
            mask = res.tile([P, 4, NC], BF16)      # diagonal masks
            ones_c = res.tile([P, 1], BF16)
            ones_r = res.tile([1, P], F32)
            negmb = res.tile([P, T], F32)          # -mean broadcast
            rstdb = res.tile([P, T], F32)          # rstd broadcast
            stat = res.tile([1, 2, T], F32)        # negmean / rstd rows
            eps_c = res.tile([1, 1], F32)

            nc.any.memset(ones_c[:], 1.0)
            nc.any.memset(ones_r[:], 1.0)
            nc.any.memset(eps_c[:], EPS)
            nc.sync.dma_start(mask[:], maskp.ap().rearrange("n p t -> p n t"))
            nc.sync.dma_start(x[:], x0t.ap().rearrange("(ko p) t -> p ko t",
                                                       p=P))

            def layernorm():
                """x (f32) -> xhat (bf16), pure normalize (scales folded)."""
                for c in range(NCH):
                    cs = slice(c * NC, (c + 1) * NC)
                    ps_s = psp.tile([1, NC], F32, tag="st")
                    ps_q = psp.tile([1, NC], F32, tag="st")
                    xbts = []
                    for k in range(KE):
                        xbt = spool.tile([P, NC], BF16, tag="xbt")
                        nc.vector.tensor_copy(out=xbt[:], in_=x[:, k, cs])
                        nc.tensor.matmul(ps_s, ones_c[:], xbt[:],
                                         start=(k == 0), stop=(k == KE - 1))
                        xbts.append(xbt)
                    for k in range(KE):
                        xsq = spool.tile([P, NC], BF16, tag="xsq")
                        nc.vector.tensor_tensor(
                            xsq[:], xbts[k][:], xbts[k][:], OP.mult)
                        nc.tensor.matmul(ps_q, ones_c[:], xsq[:],
                                         start=(k == 0), stop=(k == KE - 1))
                    t_m = spool2.tile([1, NC], F32, tag="t_m")
                    t_v = spool2.tile([1, NC], F32, tag="t_v")
                    nc.vector.tensor_scalar_mul(stat[:, 0, cs], ps_s,
                                                -1.0 / E)
                    nc.vector.tensor_scalar_mul(t_m, ps_s, 1.0 / E)
                    nc.vector.tensor_scalar_mul(t_v, ps_q, 1.0 / E)
                    nc.vector.tensor_tensor(t_m, t_m, t_m, OP.mult)
                    nc.vector.tensor_tensor(t_v, t_v, t_m, OP.subtract)
                    nc.scalar.activation(t_v, t_v, AF.Sqrt, bias=eps_c[:])
                    nc.vector.reciprocal(stat[:, 1, cs], t_v)
                    ps_b = psp.tile([P, NC], F32, tag="bc")
                    nc.tensor.matmul(ps_b, ones_r[:], stat[:, 0, cs],
                                     start=True, stop=True)
                    nc.vector.tensor_copy(out=negmb[:, cs], in_=ps_b)
                    ps_b2 = psp.tile([P, NC], F32, tag="bc")
                    nc.tensor.matmul(ps_b2, ones_r[:], stat[:, 1, cs],
                                     start=True, stop=True)
                    nc.vector.tensor_copy(out=rstdb[:, cs], in_=ps_b2)
                    for k in range(KE):
                        tmp = spool2.tile([P, NC], F32, tag="lntmp")
                        nc.vector.tensor_tensor(
                            tmp, x[:, k, cs], negmb[:, cs], OP.add)
                        nc.vector.tensor_tensor(
                            xhat[:, k, cs], tmp, rstdb[:, cs], OP.mult)

            def w6(dram_ap, m):
                """Stream a (128, KE, 128) lhsT block for output tile m."""
                wt = wst.tile([P, KE, P], BF16, tag="wm6")
                nc.sync.dma_start(
                    wt[:], dram_ap[:, m * P:(m + 1) * P].rearrange(
                        "(ko p) f -> p ko f", p=P))
                return wt

            for l in range(L):
                wv_s = wpool.tile([P, KE, E], BF16, tag="wv")
                b1_s = wpool.tile([P, KF], F32, tag="b1")
                b2_s = wpool.tile([P, KE], F32, tag="b2")
                nc.sync.dma_start(
                    wv_s[:], wv.ap()[l].rearrange("(ko p) f -> p ko f", p=P))
                nc.sync.dma_start(b1_s[:], bfc1.ap()[l])
                nc.sync.dma_start(b2_s[:], bfc2.ap()[l])

                layernorm()

                # ---- QK projection: qkT (2E, T) ----
                qk_t = apool.tile([P, 2 * KE, T], BF16, tag="qkt")
                for m in range(2 * KE):
                    wt = w6(wqk.ap()[l], m)
                    for c in range(NCH):
                        cs = slice(c * NC, (c + 1) * NC)
                        ps = psp.tile([P, NC], F32, tag="mm")
                        for k in range(KE):
                            nc.tensor.matmul(
                                ps, wt[:, k, :], xhat[:, k, cs],
                                start=(k == 0), stop=(k == KE - 1))
                        nc.vector.tensor_copy(out=qk_t[:, m, cs], in_=ps)

                # ---- V projection in (T, E) layout ----
                v_s = apool.tile([P, NT, E], BF16, tag="vs")
                for t in range(NT):
                    for (f0, fn) in ((0, NC), (NC, E - NC)):
                        ps = psp.tile([P, NC], F32, tag="mm")
                        for k in range(KE):
                            nc.tensor.matmul(
                                ps[:, :fn], xhat[:, k, t * P:(t + 1) * P],
                                wv_s[:, k, f0:f0 + fn],
                                start=(k == 0), stop=(k == KE - 1))
                        nc.vector.tensor_copy(
                            out=v_s[:, t, f0:f0 + fn], in_=ps[:, :fn])

                # ---- attention per head ----
                o_t = apool.tile([P, KE, T], BF16, tag="ot")
                for h in range(NH):
                    mt, mo = divmod(h * HD, P)
                    q_sl = qk_t[mo:mo + HD, mt, :]
                    k_sl = qk_t[mo:mo + HD, KE + mt, :]
                    for c in range(NCH):
                        cs = slice(c * NC, (c + 1) * NC)
                        ntk = 4 * (c + 1)   # causal: keep tk tiles 0..ntk-1
                        pts = []
                        for tk in range(ntk):
                            ps_s = psp.tile([P, NC], F32, tag="mm")
                            nc.tensor.matmul(
                                ps_s, k_sl[:, tk * P:(tk + 1) * P],
                                q_sl[:, cs], start=True, stop=True)
                            pt = spool.tile([P, NC], BF16, tag="pt")
                            nc.scalar.activation(pt, ps_s, AF.Exp)
                            d = tk - 4 * c
                            if d >= 0:   # diagonal-crossing tile: mask
                                nc.vector.tensor_tensor(
                                    pt, pt, mask[:, d, :], OP.mult)
                            pts.append(pt)
                        ps_o = psp.tile([P, NC], F32, tag="av")
                        ps_n = psp.tile([1, NC], F32, tag="st")
                        for i, pt in enumerate(pts):
                            nc.tensor.matmul(
                                ps_o[:HD], v_s[:, i, h * HD:(h + 1) * HD], pt,
                                start=(i == 0), stop=(i == ntk - 1))
                            nc.tensor.matmul(
                                ps_n, ones_c[:], pt,
                                start=(i == 0), stop=(i == ntk - 1))
                        rin = spool.tile([1, NC], F32, tag="rin")
                        nc.vector.reciprocal(rin, ps_n)
                        ps_r = psp.tile([P, NC], F32, tag="bc")
                        nc.tensor.matmul(ps_r[:HD], ones_r[:, :HD], rin,
                                         start=True, stop=True)
                        rb = spool.tile([P, NC], F32, tag="rb")
                        nc.vector.tensor_copy(out=rb[:HD], in_=ps_r[:HD])
                        nc.vector.tensor_tensor(
                            o_t[mo:mo + HD, mt, cs], ps_o[:HD], rb[:HD],
                            OP.mult)

                # ---- output projection + residual ----
                for m in range(KE):
                    wt = w6(wout.ap()[l], m)
                    for c in range(NCH):
                        cs = slice(c * NC, (c + 1) * NC)
                        ps = psp.tile([P, NC], F32, tag="mm")
                        for k in range(KE):
                            nc.tensor.matmul(
                                ps, wt[:, k, :], o_t[:, k, cs],
                                start=(k == 0), stop=(k == KE - 1))
                        nc.vector.tensor_tensor(
                            x[:, m, cs], ps, x[:, m, cs], OP.add)

                layernorm()

                # ---- FFN, one 512-token chunk at a time ----
                for c in range(NCH):
                    cs = slice(c * NC, (c + 1) * NC)
                    h1c = apool.tile([P, KF, NC], BF16, tag="h1c")
                    for m in range(KF):
                        wt = w6(wfc1.ap()[l], m)
                        ps = psp.tile([P, NC], F32, tag="mm")
                        for k in range(KE):
                            nc.tensor.matmul(
                                ps, wt[:, k, :], xhat[:, k, cs],
                                start=(k == 0), stop=(k == KE - 1))
                        nc.scalar.activation(
                            h1c[:, m, :], ps, AF.Gelu, bias=b1_s[:, m:m + 1])
                    for m in range(KE):
                        wt24 = wst.tile([P, KF, P], BF16, tag="wm24")
                        nc.sync.dma_start(
                            wt24[:],
                            wfc2.ap()[l][:, m * P:(m + 1) * P].rearrange(
                                "(ko p) f -> p ko f", p=P))
                        ps = psp.tile([P, NC], F32, tag="mm")
                        for k in range(KF):
                            nc.tensor.matmul(
                                ps, wt24[:, k, :], h1c[:, k, :],
                                start=(k == 0), stop=(k == KF - 1))
                        tmp = spool2.tile([P, NC], F32, tag="f2tmp")
                        nc.vector.tensor_scalar_add(tmp, ps, b2_s[:, m:m + 1])
                        nc.vector.tensor_tensor(
                            x[:, m, cs], tmp, x[:, m, cs], OP.add)

            # ---- final LN + LM head ----
            layernorm()
            for m in range(MV):
                we_m = w6(wemb.ap(), m)
                for c in range(NCH):
                    cs = slice(c * NC, (c + 1) * NC)
                    ps = psp.tile([P, NC], F32, tag="mm")
                    for k in range(KE):
                        nc.tensor.matmul(
                            ps, we_m[:, k, :], xhat[:, k, cs],
                            start=(k == 0), stop=(k == KE - 1))
                    ot = spool2.tile([P, NC], F32, tag="outsb")
                    nc.vector.tensor_copy(out=ot, in_=ps)
                    nc.sync.dma_start(out.ap()[m * P:(m + 1) * P, cs], ot)

    nc.compile()
    return nc


def _prep(inputs):
    """Host-side: fold LN scales into weights, build per-core input maps."""
    ids = np.asarray(inputs["input_ids"]).astype(np.int64)
    tok = np.asarray(inputs["tok_emb"], np.float32)
    pos = np.asarray(inputs["pos_emb"], np.float32)
    qkv = np.asarray(inputs["qkv_w"], np.float32)
    ow = np.asarray(inputs["out_w"], np.float32)
    f1 = np.asarray(inputs["fc1_w"], np.float32)
    b1 = np.asarray(inputs["fc1_b"], np.float32)
    f2 = np.asarray(inputs["fc2_w"], np.float32)
    b2 = np.asarray(inputs["fc2_b"], np.float32)
    s1 = np.asarray(inputs["ln1_scale"], np.float32)
    bb1 = np.asarray(inputs["ln1_bias"], np.float32)
    s2 = np.asarray(inputs["ln2_scale"], np.float32)
    bb2 = np.asarray(inputs["ln2_bias"], np.float32)
    sf = np.asarray(inputs["lnf_scale"], np.float32)
    bf_ = np.asarray(inputs["lnf_bias"], np.float32)
    # LN biases must be zero for the fold used here (true for this model).
    assert abs(bb1).max() == 0 and abs(bb2).max() == 0 and abs(bf_).max() == 0

    x0 = tok[ids] + pos[None, :, :]                      # (B, T, E)
    x0t = np.ascontiguousarray(x0.transpose(0, 2, 1))    # (B, E, T)

    scale = HD ** -0.5
    wqk_h = np.empty((L, E, 2 * E), BF)
    wv_h = np.empty((L, E, E), BF)
    wo_h = np.empty((L, E, E), BF)
    w1_h = np.empty((L, E, FF), BF)
    w2_h = np.empty((L, FF, E), BF)
    b1_h = np.zeros((L, P, KF), np.float32)
    b2_h = np.zeros((L, P, KE), np.float32)
    for l in range(L):
        wq = (qkv[l, :E] * s1[l][None, :]).T * scale
        wk = (qkv[l, E:2 * E] * s1[l][None, :]).T
        wv_ = (qkv[l, 2 * E:] * s1[l][None, :]).T
        wqk_h[l] = np.concatenate([wq, wk], axis=1).astype(BF)
        wv_h[l] = wv_.astype(BF)
        wo_h[l] = ow[l].T.astype(BF)
        w1_h[l] = (f1[l] * s2[l][None, :]).T.astype(BF)
        w2_h[l] = f2[l].T.astype(BF)
        b1_h[l] = b1[l].reshape(KF, P).T
        b2_h[l] = b2[l].reshape(KE, P).T

    tokp = np.zeros((4 * VP, E), np.float32)
    tokp[:V] = tok * sf[None, :]
    embt = [np.ascontiguousarray(tokp[j * VP:(j + 1) * VP].T).astype(BF)
            for j in range(4)]

    # 4 diagonal-crossing masks: d = 0,128,256,384 partition offset
    m = np.zeros((4, P, NC), np.float32)
    for i in range(4):
        gk = i * P + np.arange(P)[:, None]
        m[i] = (gk <= np.arange(NC)[None, :])
    mask_h = m.astype(BF)

    in_maps = []
    for c in range(8):
        g, j = c // 4, c % 4
        in_maps.append({
            "x0t": np.ascontiguousarray(x0t[g]),
            "wqk": wqk_h, "wv": wv_h, "wout": wo_h,
            "wfc1": w1_h, "bfc1": b1_h, "wfc2": w2_h, "bfc2": b2_h,
            "wemb": embt[j], "mask": mask_h,
        })
    return in_maps


def kernel(**inputs) -> np.ndarray:
    if "nc" not in _CACHE:
        _CACHE["nc"] = _build()
    nc = _CACHE["nc"]
    in_maps = _prep(inputs)
    res = run_bass_kernel_spmd(nc, in_maps, list(range(8)),
                               **_CACHE.get("run_kwargs", {}))
    _CACHE["last"] = res
    logits = np.empty((B, T, V), np.float32)
    for c in range(8):
        g, j = c // 4, c % 4
        lo = j * VP
        hi = min(V, lo + VP)
        logits[g, :, lo:hi] = res.results[c]["out"][:hi - lo].T
    return logits



# revision 4
# speedup vs baseline: 1.2215x; 1.2215x over previous
"""Distributed Trainium2 Bass kernel for a 4-layer GPT-style transformer.

Sharding: 8 cores = 2 batch groups x 4 vocab shards.
  - core c: batch element g = c//4, vocab shard j = c%4 (12672 ids, padded).
  - Transformer body computed per batch element (replicated within each
    group of 4); tied LM head sharded over vocab.  No collectives.

On-chip layout: activations transposed (features on partitions, tokens on
free).  LayerNorm stats via ones-matmul partition reductions; attention via
transposed scores (k @ q^T), then a second transposition in A@V: probs are
the stationary operand so the A@V output lands queries-on-partitions, with
a ones-column in V producing softmax denominators as a per-partition
column.  Normalization is then a cheap per-partition scale; a PE transpose
puts heads back features-on-partitions for the output projection.  Softmax
skips max-subtraction (|scores| < ~2 by construction); causality = 0/1 mask
multiply after exp, only on diagonal-crossing tiles.  Matmuls bf16,
residual stream fp32.  Big weight matrices stream from DRAM per out tile.
"""

import numpy as np
import ml_dtypes

import concourse.bass as bass
import concourse.mybir as mybir
import concourse.tile as tile
from concourse import bacc
from concourse.bass_utils import run_bass_kernel_spmd

V, E, NH, HD, L, T, B, FF = 50257, 768, 12, 64, 4, 1024, 2, 3072
EPS = 1e-5
P = 128
KE = E // P            # 6 feature subtiles
KF = FF // P           # 24
NT = T // P            # 8 token tiles
NC = 512               # matmul free-dim chunk
NCH = T // NC          # 2 chunks
VP = 12672             # vocab shard per core (99 * 128)
MV = VP // P           # 99
BF16 = mybir.dt.bfloat16
F32 = mybir.dt.float32
AF = mybir.ActivationFunctionType
OP = mybir.AluOpType
BF = ml_dtypes.bfloat16

_CACHE = {}


def _build():
    nc = bacc.Bacc("TRN2", target_bir_lowering=False, debug=False,
                   num_devices=8)

    x0t = nc.declare_dram_parameter("x0t", [E, T], F32, isOutput=False)
    wqk = nc.declare_dram_parameter("wqk", [L, E, 2 * E], BF16, isOutput=False)
    wv = nc.declare_dram_parameter("wv", [L, E, E], BF16, isOutput=False)
    wout = nc.declare_dram_parameter("wout", [L, E, E], BF16, isOutput=False)
    wfc1 = nc.declare_dram_parameter("wfc1", [L, E, FF], BF16, isOutput=False)
    bfc1 = nc.declare_dram_parameter("bfc1", [L, P, KF], F32, isOutput=False)
    wfc2 = nc.declare_dram_parameter("wfc2", [L, FF, E], BF16, isOutput=False)
    bfc2 = nc.declare_dram_parameter("bfc2", [L, P, KE], F32, isOutput=False)
    wemb = nc.declare_dram_parameter("wemb", [E, VP], BF16, isOutput=False)
    maskp = nc.declare_dram_parameter("mask", [4, P, NC], BF16, isOutput=False)
    identp = nc.declare_dram_parameter("ident", [P, P], BF16, isOutput=False)
    out = nc.declare_dram_parameter("out", [VP, T], F32, isOutput=True)

    with tile.TileContext(nc) as tc:
        with (
            tc.tile_pool(name="resident", bufs=1) as res,
            tc.tile_pool(name="wts", bufs=1) as wpool,
            tc.tile_pool(name="acts", bufs=1) as apool,
            tc.tile_pool(name="wstream", bufs=3) as wst,
            tc.tile_pool(name="small", bufs=3) as spool,
            tc.tile_pool(name="small2", bufs=2) as spool2,
            tc.tile_pool(name="probs", bufs=2) as ptpool,
            tc.tile_pool(name="ps", bufs=2, space="PSUM") as psp,
            tc.tile_pool(name="ps1", bufs=1, space="PSUM") as psp1,
        ):
            # --- resident tiles ---
            x = res.tile([P, KE, T], F32)          # residual stream (xT)
            xhat = res.tile([P, KE, T], BF16)      # normalized, bf16
            mask = res.tile([P, 4, NC], BF16)      # diagonal masks
            ident = res.tile([P, P], BF16)         # PE transpose identity
            v_s = res.tile([P, NT, NH, HD + 1], BF16)  # V + ones column
            ones_c = res.tile([P, 1], BF16)
            ones_r = res.tile([1, P], F32)
            negmb = res.tile([P, T], F32)          # -mean broadcast
            rstdb = res.tile([P, T], F32)          # rstd broadcast
            stat = res.tile([1, 2, T], F32)        # negmean / (rstd/E) rows
            eps_c = res.tile([1, 1], F32)

            nc.any.memset(ones_c[:], 1.0)
            nc.any.memset(ones_r[:], 1.0)
            nc.any.memset(eps_c[:], EPS)
            nc.any.memset(v_s[:, :, :, HD:HD + 1], 1.0)
            nc.sync.dma_start(mask[:], maskp.ap().rearrange("n p t -> p n t"))
            nc.sync.dma_start(ident[:], identp.ap())
            nc.sync.dma_start(x[:], x0t.ap().rearrange("(ko p) t -> p ko t",
                                                       p=P))

            def layernorm():
                """x (f32) -> xhat (bf16), pure normalize (scales folded)."""
                for c in range(NCH):
                    cs = slice(c * NC, (c + 1) * NC)
                    st = psp1.tile([1, 2, NC], F32, tag="st")
                    xbts = []
                    for k in range(KE):
                        xbt = spool.tile([P, NC], BF16, tag="xbt")
                        nc.vector.tensor_copy(out=xbt[:], in_=x[:, k, cs])
                        nc.tensor.matmul(st[:, 0, :], ones_c[:], xbt[:],
                                         start=(k == 0), stop=(k == KE - 1))
                        xbts.append(xbt)
                    for k in range(KE):
                        xsq = spool.tile([P, NC], BF16, tag="xsq")
                        nc.vector.tensor_tensor(
                            xsq[:], xbts[k][:], xbts[k][:], OP.mult)
                        nc.tensor.matmul(st[:, 1, :], ones_c[:], xsq[:],
                                         start=(k == 0), stop=(k == KE - 1))
                    # negmean row (SBUF, fp32) for the broadcast matmul
                    nc.vector.tensor_scalar_mul(stat[:, 0, cs], st[:, 0, :],
                                                -1.0 / E)
                    # var = sumsq/E - mean^2 ; rstd = 1/sqrt(var + eps)
                    sq = spool2.tile([1, NC], F32, tag="t_sq")
                    nc.vector.tensor_tensor(sq, stat[:, 0, cs], stat[:, 0, cs],
                                            OP.mult)
                    u = spool2.tile([1, NC], F32, tag="t_u")
                    nc.vector.scalar_tensor_tensor(
                        u, st[:, 1, :], 1.0 / E, sq, OP.mult, OP.subtract)
                    nc.scalar.activation(u, u, AF.Sqrt, bias=eps_c[:])
                    nc.vector.reciprocal_approx_fast(stat[:, 1, cs], u)
                    ps_b = psp.tile([P, NC], F32, tag="mm")
                    nc.tensor.matmul(ps_b, ones_r[:], stat[:, 0, cs],
                                     start=True, stop=True)
                    nc.vector.tensor_copy(out=negmb[:, cs], in_=ps_b)
                    ps_b2 = psp.tile([P, NC], F32, tag="mm")
                    nc.tensor.matmul(ps_b2, ones_r[:], stat[:, 1, cs],
                                     start=True, stop=True)
                    nc.vector.tensor_copy(out=rstdb[:, cs], in_=ps_b2)
                    for k in range(KE):
                        tmp = spool2.tile([P, NC], F32, tag="lntmp")
                        nc.vector.tensor_tensor(
                            tmp, x[:, k, cs], negmb[:, cs], OP.add)
                        nc.vector.tensor_tensor(
                            xhat[:, k, cs], tmp, rstdb[:, cs], OP.mult)

            def w6(dram_ap, m):
                """Stream a (128, KE, 128) lhsT block for output tile m."""
                wt = wst.tile([P, KE, P], BF16, tag="wm6")
                nc.sync.dma_start(
                    wt[:], dram_ap[:, m * P:(m + 1) * P].rearrange(
                        "(ko p) f -> p ko f", p=P))
                return wt

            for l in range(L):
                wv_s = wpool.tile([P, KE, E], BF16, tag="wv")
                b1_s = wpool.tile([P, KF], F32, tag="b1")
                b2_s = wpool.tile([P, KE], F32, tag="b2")
                nc.sync.dma_start(
                    wv_s[:], wv.ap()[l].rearrange("(ko p) f -> p ko f", p=P))
                nc.sync.dma_start(b1_s[:], bfc1.ap()[l])
                nc.sync.dma_start(b2_s[:], bfc2.ap()[l])

                layernorm()

                # ---- QK projection: qkT (2E, T) ----
                qk_t = apool.tile([P, 2 * KE, T], BF16, tag="qkt")
                for m in range(2 * KE):
                    wt = w6(wqk.ap()[l], m)
                    for c in range(NCH):
                        cs = slice(c * NC, (c + 1) * NC)
                        ps = psp.tile([P, NC], F32, tag="mm")
                        for k in range(KE):
                            nc.tensor.matmul(
                                ps, wt[:, k, :], xhat[:, k, cs],
                                start=(k == 0), stop=(k == KE - 1))
                        nc.vector.tensor_copy(out=qk_t[:, m, cs], in_=ps)

                # ---- V projection into (keys, head, hd+1) layout ----
                for t in range(NT):
                    for (f0, fn) in ((0, NC), (NC, E - NC)):
                        ps = psp.tile([P, NC], F32, tag="mm")
                        for k in range(KE):
                            nc.tensor.matmul(
                                ps[:, :fn], xhat[:, k, t * P:(t + 1) * P],
                                wv_s[:, k, f0:f0 + fn],
                                start=(k == 0), stop=(k == KE - 1))
                        nc.vector.tensor_copy(
                            out=v_s[:, t, f0 // HD:(f0 + fn) // HD, 0:HD],
                            in_=ps[:, :fn])

                # ---- attention, chunk-major so out-proj c=0 overlaps c=1 ---
                o_t = apool.tile([P, KE, T], BF16, tag="ot")
                for c in range(NCH):
                    cs = slice(c * NC, (c + 1) * NC)
                    ntk = 4 * (c + 1)   # causal: keep tk tiles 0..ntk-1
                    for h in range(NH):
                        mt, mo = divmod(h * HD, P)
                        q_sl = qk_t[mo:mo + HD, mt, :]
                        k_sl = qk_t[mo:mo + HD, KE + mt, :]
                        ptc = ptpool.tile([P, 8, NC], BF16, tag="ptc")
                        for tk in range(ntk):
                            ps_s = psp.tile([P, NC], F32, tag="mm")
                            nc.tensor.matmul(
                                ps_s, k_sl[:, tk * P:(tk + 1) * P],
                                q_sl[:, cs], start=True, stop=True)
                            nc.scalar.activation(ptc[:, tk, :], ps_s, AF.Exp)
                            d = tk - 4 * c
                            if d >= 0:   # diagonal-crossing tile: mask
                                nc.vector.tensor_tensor(
                                    ptc[:, tk, :], ptc[:, tk, :],
                                    mask[:, d, :], OP.mult)
                        # A@V transposed: out = probs^T @ [V | 1], so the
                        # softmax denominator lands as column HD.
                        ps_av = psp.tile([P, 4, P], F32, tag="av")
                        for tq in range(4):
                            nq = 4 * c + tq + 1
                            qs = slice(tq * P, (tq + 1) * P)
                            for i in range(nq):
                                nc.tensor.matmul(
                                    ps_av[:, tq, 0:HD + 1],
                                    ptc[:, i, qs], v_s[:, i, h, :],
                                    start=(i == 0), stop=(i == nq - 1))
                        rc = spool.tile([P, 4], F32, tag="rc")
                        nc.vector.reciprocal_approx_fast(
                            rc, ps_av[:, :, HD])
                        on = spool.tile([P, 4, HD], BF16, tag="on")
                        for tq in range(4):
                            nc.scalar.activation(
                                on[:, tq, :], ps_av[:, tq, 0:HD], AF.Copy,
                                scale=rc[:, tq:tq + 1])
                        tr = psp1.tile([HD, 4, P], BF16, tag="tr")
                        for tq in range(4):
                            nc.tensor.transpose(
                                tr[:, tq, :], on[:, tq, :], ident[:])
                        nc.vector.tensor_copy(
                            out=o_t[mo:mo + HD, mt, cs], in_=tr[:, :, :])

                # ---- output projection + residual ----
                for m in range(KE):
                    wt = w6(wout.ap()[l], m)
                    for c in range(NCH):
                        cs = slice(c * NC, (c + 1) * NC)
                        ps = psp.tile([P, NC], F32, tag="mm")
                        for k in range(KE):
                            nc.tensor.matmul(
                                ps, wt[:, k, :], o_t[:, k, cs],
                                start=(k == 0), stop=(k == KE - 1))
                        nc.vector.tensor_tensor(
                            x[:, m, cs], ps, x[:, m, cs], OP.add)

                layernorm()

                # ---- FFN, one 512-token chunk at a time ----
                for c in range(NCH):
                    cs = slice(c * NC, (c + 1) * NC)
                    h1c = apool.tile([P, KF, NC], BF16, tag="h1c")
                    for m in range(KF):
                        wt = w6(wfc1.ap()[l], m)
                        ps = psp.tile([P, NC], F32, tag="mm")
                        for k in range(KE):
                            nc.tensor.matmul(
                                ps, wt[:, k, :], xhat[:, k, cs],
                                start=(k == 0), stop=(k == KE - 1))
                        nc.scalar.activation(
                            h1c[:, m, :], ps, AF.Gelu, bias=b1_s[:, m:m + 1])
                    for m in range(KE):
                        wt24 = wst.tile([P, KF, P], BF16, tag="wm24")
                        nc.sync.dma_start(
                            wt24[:],
                            wfc2.ap()[l][:, m * P:(m + 1) * P].rearrange(
                                "(ko p) f -> p ko f", p=P))
                        ps = psp.tile([P, NC], F32, tag="mm")
                        for k in range(KF):
                            nc.tensor.matmul(
                                ps, wt24[:, k, :], h1c[:, k, :],
                                start=(k == 0), stop=(k == KF - 1))
                        nc.vector.affine_then_add(
                            x[:, m, cs], ps, x[:, m, cs],
                            scale=1.0, bias=b2_s[:, m:m + 1])

            # ---- final LN + LM head ----
            layernorm()
            for m in range(MV):
                we_m = w6(wemb.ap(), m)
                for c in range(NCH):
                    cs = slice(c * NC, (c + 1) * NC)
                    ps = psp.tile([P, NC], F32, tag="mm")
                    for k in range(KE):
                        nc.tensor.matmul(
                            ps, we_m[:, k, :], xhat[:, k, cs],
                            start=(k == 0), stop=(k == KE - 1))
                    ot = spool2.tile([P, NC], F32, tag="outsb")
                    nc.vector.tensor_copy(out=ot, in_=ps)
                    nc.sync.dma_start(out.ap()[m * P:(m + 1) * P, cs], ot)

    nc.compile()
    return nc


def _prep(inputs):
    """Host-side: fold LN scales into weights, build per-core input maps."""
    ids = np.asarray(inputs["input_ids"]).astype(np.int64)
    tok = np.asarray(inputs["tok_emb"], np.float32)
    pos = np.asarray(inputs["pos_emb"], np.float32)
    qkv = np.asarray(inputs["qkv_w"], np.float32)
    ow = np.asarray(inputs["out_w"], np.float32)
    f1 = np.asarray(inputs["fc1_w"], np.float32)
    b1 = np.asarray(inputs["fc1_b"], np.float32)
    f2 = np.asarray(inputs["fc2_w"], np.float32)
    b2 = np.asarray(inputs["fc2_b"], np.float32)
    s1 = np.asarray(inputs["ln1_scale"], np.float32)
    bb1 = np.asarray(inputs["ln1_bias"], np.float32)
    s2 = np.asarray(inputs["ln2_scale"], np.float32)
    bb2 = np.asarray(inputs["ln2_bias"], np.float32)
    sf = np.asarray(inputs["lnf_scale"], np.float32)
    bf_ = np.asarray(inputs["lnf_bias"], np.float32)
    # LN biases must be zero for the fold used here (true for this model).
    assert abs(bb1).max() == 0 and abs(bb2).max() == 0 and abs(bf_).max() == 0

    x0 = tok[ids] + pos[None, :, :]                      # (B, T, E)
    x0t = np.ascontiguousarray(x0.transpose(0, 2, 1))    # (B, E, T)

    scale = HD ** -0.5
    wqk_h = np.empty((L, E, 2 * E), BF)
    wv_h = np.empty((L, E, E), BF)
    wo_h = np.empty((L, E, E), BF)
    w1_h = np.empty((L, E, FF), BF)
    w2_h = np.empty((L, FF, E), BF)
    b1_h = np.zeros((L, P, KF), np.float32)
    b2_h = np.zeros((L, P, KE), np.float32)
    for l in range(L):
        wq = (qkv[l, :E] * s1[l][None, :]).T * scale
        wk = (qkv[l, E:2 * E] * s1[l][None, :]).T
        wv_ = (qkv[l, 2 * E:] * s1[l][None, :]).T
        wqk_h[l] = np.concatenate([wq, wk], axis=1).astype(BF)
        wv_h[l] = wv_.astype(BF)
        wo_h[l] = ow[l].T.astype(BF)
        w1_h[l] = (f1[l] * s2[l][None, :]).T.astype(BF)
        w2_h[l] = f2[l].T.astype(BF)
        b1_h[l] = b1[l].reshape(KF, P).T
        b2_h[l] = b2[l].reshape(KE, P).T

    tokp = np.zeros((4 * VP, E), np.float32)
    tokp[:V] = tok * sf[None, :]
    embt = [np.ascontiguousarray(tokp[j * VP:(j + 1) * VP].T).astype(BF)
            for j in range(4)]

    # 4 diagonal-crossing masks: d = 0,128,256,384 partition offset
    m = np.zeros((4, P, NC), np.float32)
    for i in range(4):
        gk = i * P + np.arange(P)[:, None]
        m[i] = (gk <= np.arange(NC)[None, :])
    mask_h = m.astype(BF)
    ident_h = np.eye(P, dtype=BF)

    in_maps = []
    for c in range(8):
        g, j = c // 4, c % 4
        in_maps.append({
            "x0t": np.ascontiguousarray(x0t[g]),
            "wqk": wqk_h, "wv": wv_h, "wout": wo_h,
            "wfc1": w1_h, "bfc1": b1_h, "wfc2": w2_h, "bfc2": b2_h,
            "wemb": embt[j], "mask": mask_h, "ident": ident_h,
        })
    return in_maps


def kernel(**inputs) -> np.ndarray:
    if "nc" not in _CACHE:
        _CACHE["nc"] = _build()
    nc = _CACHE["nc"]
    in_maps = _prep(inputs)
    res = run_bass_kernel_spmd(nc, in_maps, list(range(8)),
                               **_CACHE.get("run_kwargs", {}))
    _CACHE["last"] = res
    logits = np.empty((B, T, V), np.float32)
    for c in range(8):
        g, j = c // 4, c % 4
        lo = j * VP
        hi = min(V, lo + VP)
        logits[g, :, lo:hi] = res.results[c]["out"][:hi - lo].T
    return logits


# revision 5
# speedup vs baseline: 1.3516x; 1.1065x over previous
"""Distributed Trainium2 Bass kernel for a 4-layer GPT-style transformer.

Sharding: 8 cores = 2 batch groups x 4 vocab shards.
  - core c: batch element g = c//4, vocab shard j = c%4 (12672 ids, padded).
  - Transformer body computed per batch element (replicated within each
    group of 4); tied LM head sharded over vocab.  No collectives.

On-chip layout: activations transposed (features on partitions, tokens on
free).  LayerNorm stats via ones-matmul partition reductions, mean/rstd
broadcast on the (otherwise idle) GPSIMD engine; attention via transposed
scores (k @ q^T), then a second transposition in A@V: probs are the
stationary operand so the A@V output lands queries-on-partitions, with a
ones-column in V producing softmax denominators as a per-partition column.
Normalization is then a cheap per-partition scale; a PE transpose puts
heads back features-on-partitions for the output projection.  Softmax
skips max-subtraction (|scores| < ~2 by construction); causality = 0/1
mask multiply after exp, only on diagonal-crossing tiles.  The whole layer
is chunk-major (512 tokens) so chunk 0's FFN overlaps chunk 1's
exp-gated attention.  Matmuls bf16, residual stream fp32.  Big weight
matrices stream from DRAM per (chunk, out-tile).
"""

import numpy as np
import ml_dtypes

import concourse.bass as bass
import concourse.mybir as mybir
import concourse.tile as tile
from concourse import bacc
from concourse.bass_utils import run_bass_kernel_spmd

V, E, NH, HD, L, T, B, FF = 50257, 768, 12, 64, 4, 1024, 2, 3072
EPS = 1e-5
P = 128
KE = E // P            # 6 feature subtiles
KF = FF // P           # 24
NT = T // P            # 8 token tiles
NC = 512               # matmul free-dim chunk
NCH = T // NC          # 2 chunks
VP = 12672             # vocab shard per core (99 * 128)
MV = VP // P           # 99
BF16 = mybir.dt.bfloat16
F32 = mybir.dt.float32
AF = mybir.ActivationFunctionType
OP = mybir.AluOpType
BF = ml_dtypes.bfloat16

_CACHE = {}


def _build():
    nc = bacc.Bacc("TRN2", target_bir_lowering=False, debug=False,
                   num_devices=8)

    x0t = nc.declare_dram_parameter("x0t", [E, T], F32, isOutput=False)
    wqk = nc.declare_dram_parameter("wqk", [L, E, 2 * E], BF16, isOutput=False)
    wv = nc.declare_dram_parameter("wv", [L, E, E], BF16, isOutput=False)
    wout = nc.declare_dram_parameter("wout", [L, E, E], BF16, isOutput=False)
    wfc1 = nc.declare_dram_parameter("wfc1", [L, E, FF], BF16, isOutput=False)
    bfc1 = nc.declare_dram_parameter("bfc1", [L, P, KF], F32, isOutput=False)
    wfc2 = nc.declare_dram_parameter("wfc2", [L, FF, E], BF16, isOutput=False)
    bfc2 = nc.declare_dram_parameter("bfc2", [L, P, KE], F32, isOutput=False)
    wemb = nc.declare_dram_parameter("wemb", [E, VP], BF16, isOutput=False)
    maskp = nc.declare_dram_parameter("mask", [4, P, NC], BF16, isOutput=False)
    identp = nc.declare_dram_parameter("ident", [P, P], BF16, isOutput=False)
    out = nc.declare_dram_parameter("out", [VP, T], F32, isOutput=True)

    with tile.TileContext(nc) as tc:
        with (
            tc.tile_pool(name="resident", bufs=1) as res,
            tc.tile_pool(name="wts", bufs=1) as wpool,
            tc.tile_pool(name="acts", bufs=1) as apool,
            tc.tile_pool(name="wstream", bufs=3) as wst,
            tc.tile_pool(name="small", bufs=3) as spool,
            tc.tile_pool(name="small2", bufs=2) as spool2,
            tc.tile_pool(name="probs", bufs=2) as ptpool,
            tc.tile_pool(name="ps", bufs=2, space="PSUM") as psp,
            tc.tile_pool(name="ps1", bufs=1, space="PSUM") as psp1,
        ):
            # --- resident tiles ---
            x = res.tile([P, KE, T], F32)          # residual stream (xT)
            xhat = res.tile([P, KE, T], BF16)      # normalized, bf16
            mask = res.tile([P, 4, NC], BF16)      # diagonal masks
            ident = res.tile([P, P], BF16)         # PE transpose identity
            v_s = res.tile([P, NT, NH, HD + 1], BF16)  # V + ones column
            ones_c = res.tile([P, 1], BF16)
            negmb = res.tile([P, T], F32)          # -mean broadcast
            rstdb = res.tile([P, T], F32)          # rstd broadcast
            stat = res.tile([1, 2, T], F32)        # negmean / rstd rows
            eps_c = res.tile([1, 1], F32)

            nc.any.memset(ones_c[:], 1.0)
            nc.any.memset(eps_c[:], EPS)
            nc.any.memset(v_s[:, :, :, HD:HD + 1], 1.0)
            nc.sync.dma_start(mask[:], maskp.ap().rearrange("n p t -> p n t"))
            nc.sync.dma_start(ident[:], identp.ap())
            nc.sync.dma_start(x[:], x0t.ap().rearrange("(ko p) t -> p ko t",
                                                       p=P))

            def ln_chunk(c):
                """x chunk c (f32) -> xhat chunk c (bf16), scales folded."""
                cs = slice(c * NC, (c + 1) * NC)
                st = psp1.tile([1, 2, NC], F32, tag="st")
                xbts = []
                for k in range(KE):
                    xbt = spool.tile([P, NC], BF16, tag="xbt")
                    nc.vector.tensor_copy(out=xbt[:], in_=x[:, k, cs])
                    nc.tensor.matmul(st[:, 0, :], ones_c[:], xbt[:],
                                     start=(k == 0), stop=(k == KE - 1))
                    xbts.append(xbt)
                for k in range(KE):
                    xsq = spool.tile([P, NC], BF16, tag="xsq")
                    nc.vector.tensor_tensor(
                        xsq[:], xbts[k][:], xbts[k][:], OP.mult)
                    nc.tensor.matmul(st[:, 1, :], ones_c[:], xsq[:],
                                     start=(k == 0), stop=(k == KE - 1))
                # negmean row (SBUF, fp32); var = sumsq/E - mean^2
                nc.vector.tensor_scalar_mul(stat[:, 0, cs], st[:, 0, :],
                                            -1.0 / E)
                sq = spool2.tile([1, NC], F32, tag="t_sq")
                nc.vector.tensor_tensor(sq, stat[:, 0, cs], stat[:, 0, cs],
                                        OP.mult)
                u = spool2.tile([1, NC], F32, tag="t_u")
                nc.vector.scalar_tensor_tensor(
                    u, st[:, 1, :], 1.0 / E, sq, OP.mult, OP.subtract)
                nc.scalar.activation(u, u, AF.Sqrt, bias=eps_c[:])
                nc.vector.reciprocal_approx_fast(stat[:, 1, cs], u)
                # broadcast along partitions on the idle GPSIMD engine
                nc.gpsimd.partition_broadcast(negmb[:, cs], stat[:, 0, cs])
                nc.gpsimd.partition_broadcast(rstdb[:, cs], stat[:, 1, cs])
                for k in range(KE):
                    tmp = spool2.tile([P, NC], F32, tag="lntmp")
                    nc.vector.tensor_tensor(
                        tmp, x[:, k, cs], negmb[:, cs], OP.add)
                    nc.vector.tensor_tensor(
                        xhat[:, k, cs], tmp, rstdb[:, cs], OP.mult)

            def w6(dram_ap, m):
                """Stream a (128, KE, 128) lhsT block for output tile m."""
                wt = wst.tile([P, KE, P], BF16, tag="wm6")
                nc.sync.dma_start(
                    wt[:], dram_ap[:, m * P:(m + 1) * P].rearrange(
                        "(ko p) f -> p ko f", p=P))
                return wt

            for l in range(L):
                wv_s = wpool.tile([P, KE, E], BF16, tag="wv")
                b1_s = wpool.tile([P, KF], F32, tag="b1")
                b2_s = wpool.tile([P, KE], F32, tag="b2")
                nc.sync.dma_start(
                    wv_s[:], wv.ap()[l].rearrange("(ko p) f -> p ko f", p=P))
                nc.sync.dma_start(b1_s[:], bfc1.ap()[l])
                nc.sync.dma_start(b2_s[:], bfc2.ap()[l])

                # ---- LN1 + QKV + V, chunk-major ----
                qk_t = apool.tile([P, 2 * KE, T], BF16, tag="qkt")
                for c in range(NCH):
                    cs = slice(c * NC, (c + 1) * NC)
                    ln_chunk(c)
                    for m in range(2 * KE):
                        wt = w6(wqk.ap()[l], m)
                        ps = psp.tile([P, NC], F32, tag="mm")
                        for k in range(KE):
                            nc.tensor.matmul(
                                ps, wt[:, k, :], xhat[:, k, cs],
                                start=(k == 0), stop=(k == KE - 1))
                        nc.vector.tensor_copy(out=qk_t[:, m, cs], in_=ps)
                    for t in range(4 * c, 4 * c + 4):
                        for (f0, fn) in ((0, NC), (NC, E - NC)):
                            ps = psp.tile([P, NC], F32, tag="mm")
                            for k in range(KE):
                                nc.tensor.matmul(
                                    ps[:, :fn], xhat[:, k, t * P:(t + 1) * P],
                                    wv_s[:, k, f0:f0 + fn],
                                    start=(k == 0), stop=(k == KE - 1))
                            nc.vector.tensor_copy(
                                out=v_s[:, t, f0 // HD:(f0 + fn) // HD, 0:HD],
                                in_=ps[:, :fn])

                # ---- attention + out-proj + LN2 + FFN, chunk-major ----
                o_t = apool.tile([P, KE, T], BF16, tag="ot")
                for c in range(NCH):
                    cs = slice(c * NC, (c + 1) * NC)
                    ntk = 4 * (c + 1)   # causal: keep tk tiles 0..ntk-1
                    for h in range(NH):
                        mt, mo = divmod(h * HD, P)
                        q_sl = qk_t[mo:mo + HD, mt, :]
                        k_sl = qk_t[mo:mo + HD, KE + mt, :]
                        ptc = ptpool.tile([P, 8, NC], BF16, tag="ptc")
                        for tk in range(ntk):
                            ps_s = psp.tile([P, NC], F32, tag="sc")
                            nc.tensor.matmul(
                                ps_s, k_sl[:, tk * P:(tk + 1) * P],
                                q_sl[:, cs], start=True, stop=True)
                            nc.scalar.activation(ptc[:, tk, :], ps_s, AF.Exp)
                            d = tk - 4 * c
                            if d >= 0:   # diagonal-crossing tile: mask
                                nc.vector.tensor_tensor(
                                    ptc[:, tk, :], ptc[:, tk, :],
                                    mask[:, d, :], OP.mult)
                        # A@V transposed: out = probs^T @ [V | 1], so the
                        # softmax denominator lands as column HD.
                        ps_av = psp.tile([P, 4, P], F32, tag="av")
                        for tq in range(4):
                            nq = 4 * c + tq + 1
                            qs = slice(tq * P, (tq + 1) * P)
                            for i in range(nq):
                                nc.tensor.matmul(
                                    ps_av[:, tq, 0:HD + 1],
                                    ptc[:, i, qs], v_s[:, i, h, :],
                                    start=(i == 0), stop=(i == nq - 1))
                        rc = spool.tile([P, 4], F32, tag="rc")
                        nc.vector.reciprocal_approx_fast(
                            rc, ps_av[:, :, HD])
                        on = spool.tile([P, 4, HD], BF16, tag="on")
                        for tq in range(4):
                            nc.scalar.activation(
                                on[:, tq, :], ps_av[:, tq, 0:HD], AF.Copy,
                                scale=rc[:, tq:tq + 1])
                        tr = psp.tile([HD, 4, P], BF16, tag="av")
                        for tq in range(4):
                            nc.tensor.transpose(
                                tr[:, tq, :], on[:, tq, :], ident[:])
                        nc.vector.tensor_copy(
                            out=o_t[mo:mo + HD, mt, cs], in_=tr[:, :, :])

                    # ---- output projection + residual, this chunk ----
                    for m in range(KE):
                        wt = w6(wout.ap()[l], m)
                        ps = psp.tile([P, NC], F32, tag="mm")
                        for k in range(KE):
                            nc.tensor.matmul(
                                ps, wt[:, k, :], o_t[:, k, cs],
                                start=(k == 0), stop=(k == KE - 1))
                        nc.vector.tensor_tensor(
                            x[:, m, cs], ps, x[:, m, cs], OP.add)

                    ln_chunk(c)

                    # ---- FFN, this chunk ----
                    h1c = apool.tile([P, KF, NC], BF16, tag="h1c")
                    for m in range(KF):
                        wt = w6(wfc1.ap()[l], m)
                        ps = psp.tile([P, NC], F32, tag="mm")
                        for k in range(KE):
                            nc.tensor.matmul(
                                ps, wt[:, k, :], xhat[:, k, cs],
                                start=(k == 0), stop=(k == KE - 1))
                        nc.scalar.activation(
                            h1c[:, m, :], ps, AF.Gelu, bias=b1_s[:, m:m + 1])
                    for m in range(KE):
                        wt24 = wst.tile([P, KF, P], BF16, tag="wm24")
                        nc.sync.dma_start(
                            wt24[:],
                            wfc2.ap()[l][:, m * P:(m + 1) * P].rearrange(
                                "(ko p) f -> p ko f", p=P))
                        ps = psp.tile([P, NC], F32, tag="mm")
                        for k in range(KF):
                            nc.tensor.matmul(
                                ps, wt24[:, k, :], h1c[:, k, :],
                                start=(k == 0), stop=(k == KF - 1))
                        nc.vector.affine_then_add(
                            x[:, m, cs], ps, x[:, m, cs],
                            scale=1.0, bias=b2_s[:, m:m + 1])

            # ---- final LN + LM head ----
            for c in range(NCH):
                ln_chunk(c)
            for m in range(MV):
                we_m = w6(wemb.ap(), m)
                for c in range(NCH):
                    cs = slice(c * NC, (c + 1) * NC)
                    ps = psp.tile([P, NC], F32, tag="mm")
                    for k in range(KE):
                        nc.tensor.matmul(
                            ps, we_m[:, k, :], xhat[:, k, cs],
                            start=(k == 0), stop=(k == KE - 1))
                    ot = spool2.tile([P, NC], F32, tag="outsb")
                    nc.vector.tensor_copy(out=ot, in_=ps)
                    nc.sync.dma_start(out.ap()[m * P:(m + 1) * P, cs], ot)

    nc.compile()
    return nc


def _prep(inputs):
    """Host-side: fold LN scales into weights, build per-core input maps."""
    ids = np.asarray(inputs["input_ids"]).astype(np.int64)
    tok = np.asarray(inputs["tok_emb"], np.float32)
    pos = np.asarray(inputs["pos_emb"], np.float32)
    qkv = np.asarray(inputs["qkv_w"], np.float32)
    ow = np.asarray(inputs["out_w"], np.float32)
    f1 = np.asarray(inputs["fc1_w"], np.float32)
    b1 = np.asarray(inputs["fc1_b"], np.float32)
    f2 = np.asarray(inputs["fc2_w"], np.float32)
    b2 = np.asarray(inputs["fc2_b"], np.float32)
    s1 = np.asarray(inputs["ln1_scale"], np.float32)
    bb1 = np.asarray(inputs["ln1_bias"], np.float32)
    s2 = np.asarray(inputs["ln2_scale"], np.float32)
    bb2 = np.asarray(inputs["ln2_bias"], np.float32)
    sf = np.asarray(inputs["lnf_scale"], np.float32)
    bf_ = np.asarray(inputs["lnf_bias"], np.float32)
    # LN biases must be zero for the fold used here (true for this model).
    assert abs(bb1).max() == 0 and abs(bb2).max() == 0 and abs(bf_).max() == 0

    x0 = tok[ids] + pos[None, :, :]                      # (B, T, E)
    x0t = np.ascontiguousarray(x0.transpose(0, 2, 1))    # (B, E, T)

    scale = HD ** -0.5
    wqk_h = np.empty((L, E, 2 * E), BF)
    wv_h = np.empty((L, E, E), BF)
    wo_h = np.empty((L, E, E), BF)
    w1_h = np.empty((L, E, FF), BF)
    w2_h = np.empty((L, FF, E), BF)
    b1_h = np.zeros((L, P, KF), np.float32)
    b2_h = np.zeros((L, P, KE), np.float32)
    for l in range(L):
        wq = (qkv[l, :E] * s1[l][None, :]).T * scale
        wk = (qkv[l, E:2 * E] * s1[l][None, :]).T
        wv_ = (qkv[l, 2 * E:] * s1[l][None, :]).T
        wqk_h[l] = np.concatenate([wq, wk], axis=1).astype(BF)
        wv_h[l] = wv_.astype(BF)
        wo_h[l] = ow[l].T.astype(BF)
        w1_h[l] = (f1[l] * s2[l][None, :]).T.astype(BF)
        w2_h[l] = f2[l].T.astype(BF)
        b1_h[l] = b1[l].reshape(KF, P).T
        b2_h[l] = b2[l].reshape(KE, P).T

    tokp = np.zeros((4 * VP, E), np.float32)
    tokp[:V] = tok * sf[None, :]
    embt = [np.ascontiguousarray(tokp[j * VP:(j + 1) * VP].T).astype(BF)
            for j in range(4)]

    # 4 diagonal-crossing masks: d = 0,128,256,384 partition offset
    m = np.zeros((4, P, NC), np.float32)
    for i in range(4):
        gk = i * P + np.arange(P)[:, None]
        m[i] = (gk <= np.arange(NC)[None, :])
    mask_h = m.astype(BF)
    ident_h = np.eye(P, dtype=BF)

    in_maps = []
    for c in range(8):
        g, j = c // 4, c % 4
        in_maps.append({
            "x0t": np.ascontiguousarray(x0t[g]),
            "wqk": wqk_h, "wv": wv_h, "wout": wo_h,
            "wfc1": w1_h, "bfc1": b1_h, "wfc2": w2_h, "bfc2": b2_h,
            "wemb": embt[j], "mask": mask_h, "ident": ident_h,
        })
    return in_maps


def kernel(**inputs) -> np.ndarray:
    if "nc" not in _CACHE:
        _CACHE["nc"] = _build()
    nc = _CACHE["nc"]
    in_maps = _prep(inputs)
    res = run_bass_kernel_spmd(nc, in_maps, list(range(8)),
                               **_CACHE.get("run_kwargs", {}))
    _CACHE["last"] = res
    logits = np.empty((B, T, V), np.float32)
    for c in range(8):
        g, j = c // 4, c % 4
        lo = j * VP
        hi = min(V, lo + VP)
        logits[g, :, lo:hi] = res.results[c]["out"][:hi - lo].T
    return logits


# revision 7
# speedup vs baseline: 1.3730x; 1.0159x over previous
"""Distributed Trainium2 Bass kernel for a 4-layer GPT-style transformer.

Sharding: 8 cores = 2 batch groups x 4 vocab shards.
  - core c: batch element g = c//4, vocab shard j = c%4 (12672 ids, padded).
  - Transformer body computed per batch element (replicated within each
    group of 4); tied LM head sharded over vocab.  No collectives.

On-chip layout: activations transposed (features on partitions, tokens on
free).  LayerNorm stats via ones-matmul partition reductions, mean/rstd
broadcast on the (otherwise idle) GPSIMD engine; attention via transposed
scores (k @ q^T), then a second transposition in A@V: probs are the
stationary operand so the A@V output lands queries-on-partitions, with a
ones-column in V producing softmax denominators as a per-partition column.
Normalization is then a cheap per-partition scale; a PE transpose puts
heads back features-on-partitions for the output projection.  Softmax
skips max-subtraction (|scores| < ~2 by construction); causality = 0/1
mask multiply after exp, only on diagonal-crossing tiles.  The whole layer
is chunk-major (512 tokens) so chunk 0's FFN overlaps chunk 1's
exp-gated attention.  Matmuls bf16, residual stream fp32.  Big weight
matrices stream from DRAM per (chunk, out-tile).
"""

import numpy as np
import ml_dtypes

import concourse.bass as bass
import concourse.mybir as mybir
import concourse.tile as tile
from concourse import bacc
from concourse.bass_utils import run_bass_kernel_spmd

V, E, NH, HD, L, T, B, FF = 50257, 768, 12, 64, 4, 1024, 2, 3072
EPS = 1e-5
P = 128
KE = E // P            # 6 feature subtiles
KF = FF // P           # 24
NT = T // P            # 8 token tiles
NC = 512               # matmul free-dim chunk
NCH = T // NC          # 2 chunks
VP = 12672             # vocab shard per core (99 * 128)
MV = VP // P           # 99
BF16 = mybir.dt.bfloat16
F32 = mybir.dt.float32
AF = mybir.ActivationFunctionType
OP = mybir.AluOpType
BF = ml_dtypes.bfloat16

_CACHE = {}


def _build():
    nc = bacc.Bacc("TRN2", target_bir_lowering=False, debug=False,
                   num_devices=8)

    x0t = nc.declare_dram_parameter("x0t", [E, T], F32, isOutput=False)
    wqk = nc.declare_dram_parameter("wqk", [L, E, 2 * E], BF16, isOutput=False)
    wv = nc.declare_dram_parameter("wv", [L, E, E], BF16, isOutput=False)
    wout = nc.declare_dram_parameter("wout", [L, E, E], BF16, isOutput=False)
    wfc1 = nc.declare_dram_parameter("wfc1", [L, E, FF], BF16, isOutput=False)
    bfc1 = nc.declare_dram_parameter("bfc1", [L, P, KF], F32, isOutput=False)
    wfc2 = nc.declare_dram_parameter("wfc2", [L, FF, E], BF16, isOutput=False)
    bfc2 = nc.declare_dram_parameter("bfc2", [L, P, KE], F32, isOutput=False)
    wemb = nc.declare_dram_parameter("wemb", [E, VP], BF16, isOutput=False)
    maskp = nc.declare_dram_parameter("mask", [P, P], BF16, isOutput=False)
    identp = nc.declare_dram_parameter("ident", [P, P], BF16, isOutput=False)
    out = nc.declare_dram_parameter("out", [VP, T], F32, isOutput=True)

    with tile.TileContext(nc) as tc:
        with (
            tc.tile_pool(name="resident", bufs=1) as res,
            tc.tile_pool(name="wts", bufs=1) as wpool,
            tc.tile_pool(name="acts", bufs=1) as apool,
            tc.tile_pool(name="wstream", bufs=3) as wst,
            tc.tile_pool(name="small", bufs=3) as spool,
            tc.tile_pool(name="small2", bufs=2) as spool2,
            tc.tile_pool(name="probs", bufs=2) as ptpool,
            tc.tile_pool(name="ps", bufs=4, space="PSUM") as psp,
            tc.tile_pool(name="ps2", bufs=2, space="PSUM") as psp2,
        ):
            # --- resident tiles ---
            x = res.tile([P, KE, T], F32)          # residual stream (xT)
            xhat = res.tile([P, KE, T], BF16)      # normalized, bf16
            mask = res.tile([P, P], BF16)          # diagonal 0/1 block
            ident = res.tile([P, P], BF16)         # PE transpose identity
            v_s = res.tile([P, NT, NH, HD + 1], BF16)  # V + ones column
            ones_c = res.tile([P, 1], BF16)
            negmb = res.tile([P, T], F32)          # -mean broadcast
            rstdb = res.tile([P, T], F32)          # rstd broadcast
            stat = res.tile([1, 2, T], F32)        # negmean / rstd rows
            eps_c = res.tile([1, 1], F32)

            nc.any.memset(ones_c[:], 1.0)
            nc.any.memset(eps_c[:], EPS)
            nc.any.memset(v_s[:, :, :, HD:HD + 1], 1.0)
            nc.sync.dma_start(mask[:], maskp.ap())
            nc.sync.dma_start(ident[:], identp.ap())
            nc.sync.dma_start(x[:], x0t.ap().rearrange("(ko p) t -> p ko t",
                                                       p=P))

            def ln_chunk(c):
                """x chunk c (f32) -> xhat chunk c (bf16), scales folded."""
                cs = slice(c * NC, (c + 1) * NC)
                st_s = psp2.tile([1, NC], F32, tag="av")
                st_q = psp2.tile([1, NC], F32, tag="av")
                xbts = []
                for k in range(KE):
                    xbt = spool.tile([P, NC], BF16, tag="xbt")
                    nc.vector.tensor_copy(out=xbt[:], in_=x[:, k, cs])
                    nc.tensor.matmul(st_s, ones_c[:], xbt[:],
                                     start=(k == 0), stop=(k == KE - 1))
                    xbts.append(xbt)
                for k in range(KE):
                    xsq = spool.tile([P, NC], BF16, tag="xsq")
                    nc.vector.tensor_tensor(
                        xsq[:], xbts[k][:], xbts[k][:], OP.mult)
                    nc.tensor.matmul(st_q, ones_c[:], xsq[:],
                                     start=(k == 0), stop=(k == KE - 1))
                # negmean row (SBUF, fp32); var = sumsq/E - mean^2
                nc.vector.tensor_scalar_mul(stat[:, 0, cs], st_s,
                                            -1.0 / E)
                sq = spool2.tile([1, NC], F32, tag="t_sq")
                nc.vector.tensor_tensor(sq, stat[:, 0, cs], stat[:, 0, cs],
                                        OP.mult)
                u = spool2.tile([1, NC], F32, tag="t_u")
                nc.vector.scalar_tensor_tensor(
                    u, st_q, 1.0 / E, sq, OP.mult, OP.subtract)
                # rstd = exp(-0.5*ln(var+eps)): stays in the Exp act table
                nc.scalar.activation(u, u, AF.Ln, bias=eps_c[:])
                nc.scalar.activation(stat[:, 1, cs], u, AF.Exp, scale=-0.5)
                # broadcast along partitions on the idle GPSIMD engine
                nc.gpsimd.partition_broadcast(negmb[:, cs], stat[:, 0, cs])
                nc.gpsimd.partition_broadcast(rstdb[:, cs], stat[:, 1, cs])
                for k in range(KE):
                    tmp = spool2.tile([P, NC], F32, tag="lntmp")
                    nc.vector.tensor_tensor(
                        tmp, x[:, k, cs], negmb[:, cs], OP.add)
                    nc.vector.tensor_tensor(
                        xhat[:, k, cs], tmp, rstdb[:, cs], OP.mult)

            def w6(dram_ap, m):
                """Stream a (128, KE, 128) lhsT block for output tile m."""
                wt = wst.tile([P, KE, P], BF16, tag="wm6")
                nc.sync.dma_start(
                    wt[:], dram_ap[:, m * P:(m + 1) * P].rearrange(
                        "(ko p) f -> p ko f", p=P))
                return wt

            for l in range(L):
                wv_s = wpool.tile([P, KE, E], BF16, tag="wv")
                b1_s = wpool.tile([P, KF], F32, tag="b1")
                b2_s = wpool.tile([P, KE], F32, tag="b2")
                nc.sync.dma_start(
                    wv_s[:], wv.ap()[l].rearrange("(ko p) f -> p ko f", p=P))
                nc.sync.dma_start(b1_s[:], bfc1.ap()[l])
                nc.sync.dma_start(b2_s[:], bfc2.ap()[l])

                # ---- LN1 + QKV + V, chunk-major ----
                qk_t = apool.tile([P, 2 * KE, T], BF16, tag="qkt")
                for c in range(NCH):
                    cs = slice(c * NC, (c + 1) * NC)
                    ln_chunk(c)
                    for m in range(2 * KE):
                        wt = w6(wqk.ap()[l], m)
                        ps = psp.tile([P, NC], F32, tag="mm")
                        for k in range(KE):
                            nc.tensor.matmul(
                                ps, wt[:, k, :], xhat[:, k, cs],
                                start=(k == 0), stop=(k == KE - 1))
                        nc.vector.tensor_copy(out=qk_t[:, m, cs], in_=ps)
                    for t in range(4 * c, 4 * c + 4):
                        for (f0, fn) in ((0, NC), (NC, E - NC)):
                            ps = psp.tile([P, NC], F32, tag="mm")
                            for k in range(KE):
                                nc.tensor.matmul(
                                    ps[:, :fn], xhat[:, k, t * P:(t + 1) * P],
                                    wv_s[:, k, f0:f0 + fn],
                                    start=(k == 0), stop=(k == KE - 1))
                            nc.vector.tensor_copy(
                                out=v_s[:, t, f0 // HD:(f0 + fn) // HD, 0:HD],
                                in_=ps[:, :fn])

                # ---- attention + out-proj + LN2 + FFN, chunk-major ----
                o_t = apool.tile([P, KE, T], BF16, tag="ot")
                for c in range(NCH):
                    cs = slice(c * NC, (c + 1) * NC)
                    ntk = 4 * (c + 1)   # causal: keep tk tiles 0..ntk-1
                    for h in range(NH):
                        mt, mo = divmod(h * HD, P)
                        q_sl = qk_t[mo:mo + HD, mt, :]
                        k_sl = qk_t[mo:mo + HD, KE + mt, :]
                        ptc = ptpool.tile([P, 8, NC], BF16, tag="ptc")
                        for tk in range(ntk):
                            d = tk - 4 * c
                            d0 = max(d, 0) * P   # cols < d0 are fully masked
                            ps_s = psp2.tile([P, NC], F32, tag="sc")
                            nc.tensor.matmul(
                                ps_s[:, d0:], k_sl[:, tk * P:(tk + 1) * P],
                                q_sl[:, c * NC + d0:(c + 1) * NC],
                                start=True, stop=True)
                            nc.scalar.activation(ptc[:, tk, d0:],
                                                 ps_s[:, d0:], AF.Exp)
                            if d >= 0:   # diagonal block: triangular mask
                                nc.vector.tensor_tensor(
                                    ptc[:, tk, d0:d0 + P],
                                    ptc[:, tk, d0:d0 + P],
                                    mask[:], OP.mult)
                        # A@V transposed: out = probs^T @ [V | 1], so the
                        # softmax denominator lands as column HD.
                        ps_av = psp2.tile([P, 4, P], F32, tag="av")
                        for tq in range(4):
                            nq = 4 * c + tq + 1
                            qs = slice(tq * P, (tq + 1) * P)
                            for i in range(nq):
                                nc.tensor.matmul(
                                    ps_av[:, tq, 0:HD + 1],
                                    ptc[:, i, qs], v_s[:, i, h, :],
                                    start=(i == 0), stop=(i == nq - 1))
                        rc = spool.tile([P, 4], F32, tag="rc")
                        nc.vector.reciprocal_approx_fast(
                            rc, ps_av[:, :, HD])
                        on = spool.tile([P, 4, HD], BF16, tag="on")
                        for tq in range(4):
                            nc.scalar.activation(
                                on[:, tq, :], ps_av[:, tq, 0:HD], AF.Copy,
                                scale=rc[:, tq:tq + 1])
                        tr = psp2.tile([HD, 4, P], BF16, tag="av")
                        for tq in range(4):
                            nc.tensor.transpose(
                                tr[:, tq, :], on[:, tq, :], ident[:])
                        nc.vector.tensor_copy(
                            out=o_t[mo:mo + HD, mt, cs], in_=tr[:, :, :])

                    # ---- output projection + residual, this chunk ----
                    for m in range(KE):
                        wt = w6(wout.ap()[l], m)
                        ps = psp.tile([P, NC], F32, tag="mm")
                        for k in range(KE):
                            nc.tensor.matmul(
                                ps, wt[:, k, :], o_t[:, k, cs],
                                start=(k == 0), stop=(k == KE - 1))
                        nc.vector.tensor_tensor(
                            x[:, m, cs], ps, x[:, m, cs], OP.add)

                    ln_chunk(c)

                    # ---- FFN, this chunk ----
                    h1c = apool.tile([P, KF, NC], BF16, tag="h1c")
                    for m in range(KF):
                        wt = w6(wfc1.ap()[l], m)
                        ps = psp.tile([P, NC], F32, tag="mm")
                        for k in range(KE):
                            nc.tensor.matmul(
                                ps, wt[:, k, :], xhat[:, k, cs],
                                start=(k == 0), stop=(k == KE - 1))
                        nc.scalar.activation(
                            h1c[:, m, :], ps, AF.Gelu, bias=b1_s[:, m:m + 1])
                    for m in range(KE):
                        wt24 = wst.tile([P, KF, P], BF16, tag="wm24")
                        nc.sync.dma_start(
                            wt24[:],
                            wfc2.ap()[l][:, m * P:(m + 1) * P].rearrange(
                                "(ko p) f -> p ko f", p=P))
                        ps = psp.tile([P, NC], F32, tag="mm")
                        for k in range(KF):
                            nc.tensor.matmul(
                                ps, wt24[:, k, :], h1c[:, k, :],
                                start=(k == 0), stop=(k == KF - 1))
                        nc.vector.affine_then_add(
                            x[:, m, cs], ps, x[:, m, cs],
                            scale=1.0, bias=b2_s[:, m:m + 1])

            # ---- final LN + LM head ----
            for c in range(NCH):
                ln_chunk(c)
            for m in range(MV):
                we_m = w6(wemb.ap(), m)
                for c in range(NCH):
                    cs = slice(c * NC, (c + 1) * NC)
                    ps = psp.tile([P, NC], F32, tag="mm")
                    for k in range(KE):
                        nc.tensor.matmul(
                            ps, we_m[:, k, :], xhat[:, k, cs],
                            start=(k == 0), stop=(k == KE - 1))
                    ot = spool2.tile([P, NC], F32, tag="outsb")
                    nc.vector.tensor_copy(out=ot, in_=ps)
                    nc.sync.dma_start(out.ap()[m * P:(m + 1) * P, cs], ot)

    nc.compile()
    return nc


def _prep(inputs):
    """Host-side: fold LN scales into weights, build per-core input maps."""
    ids = np.asarray(inputs["input_ids"]).astype(np.int64)
    tok = np.asarray(inputs["tok_emb"], np.float32)
    pos = np.asarray(inputs["pos_emb"], np.float32)
    qkv = np.asarray(inputs["qkv_w"], np.float32)
    ow = np.asarray(inputs["out_w"], np.float32)
    f1 = np.asarray(inputs["fc1_w"], np.float32)
    b1 = np.asarray(inputs["fc1_b"], np.float32)
    f2 = np.asarray(inputs["fc2_w"], np.float32)
    b2 = np.asarray(inputs["fc2_b"], np.float32)
    s1 = np.asarray(inputs["ln1_scale"], np.float32)
    bb1 = np.asarray(inputs["ln1_bias"], np.float32)
    s2 = np.asarray(inputs["ln2_scale"], np.float32)
    bb2 = np.asarray(inputs["ln2_bias"], np.float32)
    sf = np.asarray(inputs["lnf_scale"], np.float32)
    bf_ = np.asarray(inputs["lnf_bias"], np.float32)
    # LN biases must be zero for the fold used here (true for this model).
    assert abs(bb1).max() == 0 and abs(bb2).max() == 0 and abs(bf_).max() == 0

    x0 = tok[ids] + pos[None, :, :]                      # (B, T, E)
    x0t = np.ascontiguousarray(x0.transpose(0, 2, 1))    # (B, E, T)

    scale = HD ** -0.5
    wqk_h = np.empty((L, E, 2 * E), BF)
    wv_h = np.empty((L, E, E), BF)
    wo_h = np.empty((L, E, E), BF)
    w1_h = np.empty((L, E, FF), BF)
    w2_h = np.empty((L, FF, E), BF)
    b1_h = np.zeros((L, P, KF), np.float32)
    b2_h = np.zeros((L, P, KE), np.float32)
    for l in range(L):
        wq = (qkv[l, :E] * s1[l][None, :]).T * scale
        wk = (qkv[l, E:2 * E] * s1[l][None, :]).T
        wv_ = (qkv[l, 2 * E:] * s1[l][None, :]).T
        wqk_h[l] = np.concatenate([wq, wk], axis=1).astype(BF)
        wv_h[l] = wv_.astype(BF)
        wo_h[l] = ow[l].T.astype(BF)
        w1_h[l] = (f1[l] * s2[l][None, :]).T.astype(BF)
        w2_h[l] = f2[l].T.astype(BF)
        b1_h[l] = b1[l].reshape(KF, P).T
        b2_h[l] = b2[l].reshape(KE, P).T

    tokp = np.zeros((4 * VP, E), np.float32)
    tokp[:V] = tok * sf[None, :]
    embt = [np.ascontiguousarray(tokp[j * VP:(j + 1) * VP].T).astype(BF)
            for j in range(4)]

    # lower-triangular (inclusive) 0/1 block for the diagonal tiles
    mask_h = (np.arange(P)[:, None] <= np.arange(P)[None, :]).astype(BF)
    ident_h = np.eye(P, dtype=BF)

    in_maps = []
    for c in range(8):
        g, j = c // 4, c % 4
        in_maps.append({
            "x0t": np.ascontiguousarray(x0t[g]),
            "wqk": wqk_h, "wv": wv_h, "wout": wo_h,
            "wfc1": w1_h, "bfc1": b1_h, "wfc2": w2_h, "bfc2": b2_h,
            "wemb": embt[j], "mask": mask_h, "ident": ident_h,
        })
    return in_maps


def kernel(**inputs) -> np.ndarray:
    if "nc" not in _CACHE:
        _CACHE["nc"] = _build()
    nc = _CACHE["nc"]
    in_maps = _prep(inputs)
    res = run_bass_kernel_spmd(nc, in_maps, list(range(8)),
                               **_CACHE.get("run_kwargs", {}))
    _CACHE["last"] = res
    logits = np.empty((B, T, V), np.float32)
    for c in range(8):
        g, j = c // 4, c % 4
        lo = j * VP
        hi = min(V, lo + VP)
        logits[g, :, lo:hi] = res.results[c]["out"][:hi - lo].T
    return logits


# revision 8
# speedup vs baseline: 1.4890x; 1.0845x over previous
"""Distributed Trainium2 Bass kernel for a 4-layer GPT-style transformer.

Sharding: 8 cores = 2 batch groups x 4 vocab shards.
  - core c: batch element g = c//4, vocab shard j = c%4 (12672 ids, padded).
  - Transformer body computed per batch element (replicated within each
    group of 4); tied LM head sharded over vocab.  No collectives.

On-chip layout: activations transposed (features on partitions, tokens on
free).  LayerNorm stats via ones-matmul partition reductions, mean/rstd
broadcast on the (otherwise idle) GPSIMD engine; attention via transposed
scores (k @ q^T), then a second transposition in A@V: probs are the
stationary operand so the A@V output lands queries-on-partitions, with a
ones-column in V producing softmax denominators as a per-partition column.
Normalization is then a cheap per-partition scale; a PE transpose puts
heads back features-on-partitions for the output projection.  Softmax
skips max-subtraction (|scores| < ~2 by construction); causality = 0/1
mask multiply after exp, only on diagonal-crossing tiles.  The whole layer
is chunk-major (512 tokens) so chunk 0's FFN overlaps chunk 1's
exp-gated attention.  Matmuls bf16, residual stream fp32.  Big weight
matrices stream from DRAM per (chunk, out-tile).
"""

import numpy as np
import ml_dtypes

import concourse.bass as bass
import concourse.mybir as mybir
import concourse.tile as tile
from concourse import bacc
from concourse.bass_utils import run_bass_kernel_spmd

V, E, NH, HD, L, T, B, FF = 50257, 768, 12, 64, 4, 1024, 2, 3072
EPS = 1e-5
P = 128
KE = E // P            # 6 feature subtiles
KF = FF // P           # 24
NT = T // P            # 8 token tiles
NC = 512               # matmul free-dim chunk
NCH = T // NC          # 2 chunks
VP = 12672             # vocab shard per core (99 * 128)
MV = VP // P           # 99
BF16 = mybir.dt.bfloat16
F32 = mybir.dt.float32
AF = mybir.ActivationFunctionType
OP = mybir.AluOpType
BF = ml_dtypes.bfloat16

_CACHE = {}


def _build():
    nc = bacc.Bacc("TRN2", target_bir_lowering=False, debug=False,
                   num_devices=8)

    x0t = nc.declare_dram_parameter("x0t", [E, T], F32, isOutput=False)
    wqk = nc.declare_dram_parameter("wqk", [L, E, 2 * E], BF16, isOutput=False)
    wv = nc.declare_dram_parameter("wv", [L, E, E], BF16, isOutput=False)
    wout = nc.declare_dram_parameter("wout", [L, E, E], BF16, isOutput=False)
    wfc1 = nc.declare_dram_parameter("wfc1", [L, E, FF], BF16, isOutput=False)
    bfc1 = nc.declare_dram_parameter("bfc1", [L, P, KF], F32, isOutput=False)
    wfc2 = nc.declare_dram_parameter("wfc2", [L, FF, E], BF16, isOutput=False)
    bfc2 = nc.declare_dram_parameter("bfc2", [L, P, KE], F32, isOutput=False)
    wemb = nc.declare_dram_parameter("wemb", [E, VP], BF16, isOutput=False)
    maskp = nc.declare_dram_parameter("mask", [P, P], BF16, isOutput=False)
    identp = nc.declare_dram_parameter("ident", [P, P], BF16, isOutput=False)
    out = nc.declare_dram_parameter("out", [VP, T], F32, isOutput=True)

    with tile.TileContext(nc) as tc:
        with (
            tc.tile_pool(name="resident", bufs=1) as res,
            tc.tile_pool(name="wts", bufs=1) as wpool,
            tc.tile_pool(name="acts", bufs=1) as apool,
            tc.tile_pool(name="wstream", bufs=8) as wst,
            tc.tile_pool(name="wstream24", bufs=3) as wst24,
            tc.tile_pool(name="small", bufs=3) as spool,
            tc.tile_pool(name="small2", bufs=2) as spool2,
            tc.tile_pool(name="probs", bufs=2) as ptpool,
            tc.tile_pool(name="ps", bufs=4, space="PSUM") as psp,
            tc.tile_pool(name="ps2", bufs=2, space="PSUM") as psp2,
        ):
            # --- resident tiles ---
            x = res.tile([P, KE, T], F32)          # residual stream (xT)
            xhat = res.tile([P, KE, T], BF16)      # normalized, bf16
            mask = res.tile([P, P], BF16)          # diagonal 0/1 block
            ident = res.tile([P, P], BF16)         # PE transpose identity
            v_s = res.tile([P, NT, NH, HD + 1], BF16)  # V + ones column
            ones_c = res.tile([P, 1], BF16)
            negmb = res.tile([P, T], F32)          # -mean broadcast
            rstdb = res.tile([P, T], F32)          # rstd broadcast
            stat = res.tile([1, 2, T], F32)        # negmean / rstd rows
            eps_c = res.tile([1, 1], F32)

            nc.any.memset(ones_c[:], 1.0)
            nc.any.memset(eps_c[:], EPS)
            nc.any.memset(v_s[:, :, :, HD:HD + 1], 1.0)
            nc.sync.dma_start(mask[:], maskp.ap())
            nc.sync.dma_start(ident[:], identp.ap())
            nc.sync.dma_start(x[:], x0t.ap().rearrange("(ko p) t -> p ko t",
                                                       p=P))

            def ln_chunk(c):
                """x chunk c (f32) -> xhat chunk c (bf16), scales folded."""
                cs = slice(c * NC, (c + 1) * NC)
                st_s = psp2.tile([1, NC], F32, tag="av")
                st_q = psp2.tile([1, NC], F32, tag="av")
                xbts = []
                for k in range(KE):
                    xbt = spool.tile([P, NC], BF16, tag="xbt")
                    nc.vector.tensor_copy(out=xbt[:], in_=x[:, k, cs])
                    nc.tensor.matmul(st_s, ones_c[:], xbt[:],
                                     start=(k == 0), stop=(k == KE - 1))
                    xbts.append(xbt)
                for k in range(KE):
                    xsq = spool.tile([P, NC], BF16, tag="xsq")
                    nc.vector.tensor_tensor(
                        xsq[:], xbts[k][:], xbts[k][:], OP.mult)
                    nc.tensor.matmul(st_q, ones_c[:], xsq[:],
                                     start=(k == 0), stop=(k == KE - 1))
                # negmean row (SBUF, fp32); var = sumsq/E - mean^2
                nc.vector.tensor_scalar_mul(stat[:, 0, cs], st_s,
                                            -1.0 / E)
                sq = spool2.tile([1, NC], F32, tag="t_sq")
                nc.vector.tensor_tensor(sq, stat[:, 0, cs], stat[:, 0, cs],
                                        OP.mult)
                u = spool2.tile([1, NC], F32, tag="t_u")
                nc.vector.scalar_tensor_tensor(
                    u, st_q, 1.0 / E, sq, OP.mult, OP.subtract)
                # rstd = exp(-0.5*ln(var+eps)): stays in the Exp act table
                nc.scalar.activation(u, u, AF.Ln, bias=eps_c[:])
                nc.scalar.activation(stat[:, 1, cs], u, AF.Exp, scale=-0.5)
                # broadcast along partitions on the idle GPSIMD engine
                nc.gpsimd.partition_broadcast(negmb[:, cs], stat[:, 0, cs])
                nc.gpsimd.partition_broadcast(rstdb[:, cs], stat[:, 1, cs])
                for k in range(KE):
                    tmp = spool2.tile([P, NC], F32, tag="lntmp")
                    nc.vector.tensor_tensor(
                        tmp, x[:, k, cs], negmb[:, cs], OP.add)
                    nc.vector.tensor_tensor(
                        xhat[:, k, cs], tmp, rstdb[:, cs], OP.mult)

            def w6(dram_ap, m):
                """Stream a (128, KE, 128) lhsT block for output tile m."""
                wt = wst.tile([P, KE, P], BF16, tag="wm6")
                nc.sync.dma_start(
                    wt[:], dram_ap[:, m * P:(m + 1) * P].rearrange(
                        "(ko p) f -> p ko f", p=P))
                return wt

            for l in range(L):
                wv_s = wpool.tile([P, KE, E], BF16, tag="wv")
                b1_s = wpool.tile([P, KF], F32, tag="b1")
                b2_s = wpool.tile([P, KE], F32, tag="b2")
                nc.sync.dma_start(
                    wv_s[:], wv.ap()[l].rearrange("(ko p) f -> p ko f", p=P))
                nc.sync.dma_start(b1_s[:], bfc1.ap()[l])
                nc.sync.dma_start(b2_s[:], bfc2.ap()[l])

                # ---- LN1 + QKV + V, chunk-major ----
                qk_t = apool.tile([P, 2 * KE, T], BF16, tag="qkt")
                for c in range(NCH):
                    cs = slice(c * NC, (c + 1) * NC)
                    ln_chunk(c)
                    for m in range(2 * KE):
                        wt = w6(wqk.ap()[l], m)
                        ps = psp.tile([P, NC], F32, tag="mm")
                        for k in range(KE):
                            nc.tensor.matmul(
                                ps, wt[:, k, :], xhat[:, k, cs],
                                start=(k == 0), stop=(k == KE - 1))
                        nc.vector.tensor_copy(out=qk_t[:, m, cs], in_=ps)
                    for t in range(4 * c, 4 * c + 4):
                        for (f0, fn) in ((0, NC), (NC, E - NC)):
                            ps = psp.tile([P, NC], F32, tag="mm")
                            for k in range(KE):
                                nc.tensor.matmul(
                                    ps[:, :fn], xhat[:, k, t * P:(t + 1) * P],
                                    wv_s[:, k, f0:f0 + fn],
                                    start=(k == 0), stop=(k == KE - 1))
                            nc.vector.tensor_copy(
                                out=v_s[:, t, f0 // HD:(f0 + fn) // HD, 0:HD],
                                in_=ps[:, :fn])

                # ---- attention + out-proj + LN2 + FFN, chunk-major ----
                o_t = apool.tile([P, KE, T], BF16, tag="ot")
                for c in range(NCH):
                    cs = slice(c * NC, (c + 1) * NC)
                    ntk = 4 * (c + 1)   # causal: keep tk tiles 0..ntk-1
                    for h in range(NH):
                        mt, mo = divmod(h * HD, P)
                        q_sl = qk_t[mo:mo + HD, mt, :]
                        k_sl = qk_t[mo:mo + HD, KE + mt, :]
                        ptc = ptpool.tile([P, 8, NC], BF16, tag="ptc")
                        for tk in range(ntk):
                            d = tk - 4 * c
                            d0 = max(d, 0) * P   # cols < d0 are fully masked
                            ps_s = psp2.tile([P, NC], F32, tag="sc")
                            nc.tensor.matmul(
                                ps_s[:, d0:], k_sl[:, tk * P:(tk + 1) * P],
                                q_sl[:, c * NC + d0:(c + 1) * NC],
                                start=True, stop=True)
                            nc.scalar.activation(ptc[:, tk, d0:],
                                                 ps_s[:, d0:], AF.Exp)
                            if d >= 0:   # diagonal block: triangular mask
                                nc.vector.tensor_tensor(
                                    ptc[:, tk, d0:d0 + P],
                                    ptc[:, tk, d0:d0 + P],
                                    mask[:], OP.mult)
                        # A@V transposed: out = probs^T @ [V | 1], so the
                        # softmax denominator lands as column HD.
                        ps_av = psp2.tile([P, 4, P], F32, tag="av")
                        for tq in range(4):
                            nq = 4 * c + tq + 1
                            qs = slice(tq * P, (tq + 1) * P)
                            for i in range(nq):
                                nc.tensor.matmul(
                                    ps_av[:, tq, 0:HD + 1],
                                    ptc[:, i, qs], v_s[:, i, h, :],
                                    start=(i == 0), stop=(i == nq - 1))
                        rc = spool.tile([P, 4], F32, tag="rc")
                        nc.vector.reciprocal_approx_fast(
                            rc, ps_av[:, :, HD])
                        on = spool.tile([P, 4, HD], BF16, tag="on")
                        for tq in range(4):
                            nc.vector.tensor_scalar_mul(
                                on[:, tq, :], ps_av[:, tq, 0:HD],
                                rc[:, tq:tq + 1])
                        tr = psp2.tile([HD, 4, P], BF16, tag="av")
                        for tq in range(4):
                            nc.tensor.transpose(
                                tr[:, tq, :], on[:, tq, :], ident[:])
                        nc.vector.tensor_copy(
                            out=o_t[mo:mo + HD, mt, cs], in_=tr[:, :, :])

                    # ---- output projection + residual, this chunk ----
                    for m in range(KE):
                        wt = w6(wout.ap()[l], m)
                        ps = psp.tile([P, NC], F32, tag="mm")
                        for k in range(KE):
                            nc.tensor.matmul(
                                ps, wt[:, k, :], o_t[:, k, cs],
                                start=(k == 0), stop=(k == KE - 1))
                        nc.vector.tensor_tensor(
                            x[:, m, cs], ps, x[:, m, cs], OP.add)

                    ln_chunk(c)

                    # ---- FFN, this chunk ----
                    h1c = apool.tile([P, KF, NC], BF16, tag="h1c")
                    for m in range(KF):
                        wt = w6(wfc1.ap()[l], m)
                        ps = psp.tile([P, NC], F32, tag="mm")
                        for k in range(KE):
                            nc.tensor.matmul(
                                ps, wt[:, k, :], xhat[:, k, cs],
                                start=(k == 0), stop=(k == KE - 1))
                        nc.scalar.activation(
                            h1c[:, m, :], ps, AF.Gelu, bias=b1_s[:, m:m + 1])
                    for m in range(KE):
                        wt24 = wst24.tile([P, KF, P], BF16, tag="wm24")
                        nc.sync.dma_start(
                            wt24[:],
                            wfc2.ap()[l][:, m * P:(m + 1) * P].rearrange(
                                "(ko p) f -> p ko f", p=P))
                        ps = psp.tile([P, NC], F32, tag="mm")
                        for k in range(KF):
                            nc.tensor.matmul(
                                ps, wt24[:, k, :], h1c[:, k, :],
                                start=(k == 0), stop=(k == KF - 1))
                        nc.vector.affine_then_add(
                            x[:, m, cs], ps, x[:, m, cs],
                            scale=1.0, bias=b2_s[:, m:m + 1])

            # ---- final LN + LM head ----
            for c in range(NCH):
                ln_chunk(c)
            for m in range(MV):
                we_m = w6(wemb.ap(), m)
                for c in range(NCH):
                    cs = slice(c * NC, (c + 1) * NC)
                    ps = psp.tile([P, NC], F32, tag="mm")
                    for k in range(KE):
                        nc.tensor.matmul(
                            ps, we_m[:, k, :], xhat[:, k, cs],
                            start=(k == 0), stop=(k == KE - 1))
                    ot = spool2.tile([P, NC], F32, tag="outsb")
                    nc.vector.tensor_copy(out=ot, in_=ps)
                    nc.sync.dma_start(out.ap()[m * P:(m + 1) * P, cs], ot)

    nc.compile()
    return nc


def _prep(inputs):
    """Host-side: fold LN scales into weights, build per-core input maps."""
    ids = np.asarray(inputs["input_ids"]).astype(np.int64)
    tok = np.asarray(inputs["tok_emb"], np.float32)
    pos = np.asarray(inputs["pos_emb"], np.float32)
    qkv = np.asarray(inputs["qkv_w"], np.float32)
    ow = np.asarray(inputs["out_w"], np.float32)
    f1 = np.asarray(inputs["fc1_w"], np.float32)
    b1 = np.asarray(inputs["fc1_b"], np.float32)
    f2 = np.asarray(inputs["fc2_w"], np.float32)
    b2 = np.asarray(inputs["fc2_b"], np.float32)
    s1 = np.asarray(inputs["ln1_scale"], np.float32)
    bb1 = np.asarray(inputs["ln1_bias"], np.float32)
    s2 = np.asarray(inputs["ln2_scale"], np.float32)
    bb2 = np.asarray(inputs["ln2_bias"], np.float32)
    sf = np.asarray(inputs["lnf_scale"], np.float32)
    bf_ = np.asarray(inputs["lnf_bias"], np.float32)
    # LN biases must be zero for the fold used here (true for this model).
    assert abs(bb1).max() == 0 and abs(bb2).max() == 0 and abs(bf_).max() == 0

    x0 = tok[ids] + pos[None, :, :]                      # (B, T, E)
    x0t = np.ascontiguousarray(x0.transpose(0, 2, 1))    # (B, E, T)

    scale = HD ** -0.5
    wqk_h = np.empty((L, E, 2 * E), BF)
    wv_h = np.empty((L, E, E), BF)
    wo_h = np.empty((L, E, E), BF)
    w1_h = np.empty((L, E, FF), BF)
    w2_h = np.empty((L, FF, E), BF)
    b1_h = np.zeros((L, P, KF), np.float32)
    b2_h = np.zeros((L, P, KE), np.float32)
    for l in range(L):
        wq = (qkv[l, :E] * s1[l][None, :]).T * scale
        wk = (qkv[l, E:2 * E] * s1[l][None, :]).T
        wv_ = (qkv[l, 2 * E:] * s1[l][None, :]).T
        wqk_h[l] = np.concatenate([wq, wk], axis=1).astype(BF)
        wv_h[l] = wv_.astype(BF)
        wo_h[l] = ow[l].T.astype(BF)
        w1_h[l] = (f1[l] * s2[l][None, :]).T.astype(BF)
        w2_h[l] = f2[l].T.astype(BF)
        b1_h[l] = b1[l].reshape(KF, P).T
        b2_h[l] = b2[l].reshape(KE, P).T

    tokp = np.zeros((4 * VP, E), np.float32)
    tokp[:V] = tok * sf[None, :]
    embt = [np.ascontiguousarray(tokp[j * VP:(j + 1) * VP].T).astype(BF)
            for j in range(4)]

    # lower-triangular (inclusive) 0/1 block for the diagonal tiles
    mask_h = (np.arange(P)[:, None] <= np.arange(P)[None, :]).astype(BF)
    ident_h = np.eye(P, dtype=BF)

    in_maps = []
    for c in range(8):
        g, j = c // 4, c % 4
        in_maps.append({
            "x0t": np.ascontiguousarray(x0t[g]),
            "wqk": wqk_h, "wv": wv_h, "wout": wo_h,
            "wfc1": w1_h, "bfc1": b1_h, "wfc2": w2_h, "bfc2": b2_h,
            "wemb": embt[j], "mask": mask_h, "ident": ident_h,
        })
    return in_maps


def kernel(**inputs) -> np.ndarray:
    if "nc" not in _CACHE:
        _CACHE["nc"] = _build()
    nc = _CACHE["nc"]
    in_maps = _prep(inputs)
    res = run_bass_kernel_spmd(nc, in_maps, list(range(8)),
                               **_CACHE.get("run_kwargs", {}))
    _CACHE["last"] = res
    logits = np.empty((B, T, V), np.float32)
    for c in range(8):
        g, j = c // 4, c % 4
        lo = j * VP
        hi = min(V, lo + VP)
        logits[g, :, lo:hi] = res.results[c]["out"][:hi - lo].T
    return logits


# revision 10
# speedup vs baseline: 1.4897x; 1.0004x over previous
"""Distributed Trainium2 Bass kernel for a 4-layer GPT-style transformer.

Sharding: 8 cores = 2 batch groups x 4 vocab shards.
  - core c: batch element g = c//4, vocab shard j = c%4 (12672 ids, padded).
  - Transformer body computed per batch element (replicated within each
    group of 4); tied LM head sharded over vocab.  No collectives.

On-chip layout: activations transposed (features on partitions, tokens on
free).  LayerNorm stats via ones-matmul partition reductions, mean/rstd
broadcast on the (otherwise idle) GPSIMD engine; attention via transposed
scores (k @ q^T), then a second transposition in A@V: probs are the
stationary operand so the A@V output lands queries-on-partitions, with a
ones-column in V producing softmax denominators as a per-partition column.
Normalization is then a cheap per-partition scale; a PE transpose puts
heads back features-on-partitions for the output projection.  Softmax
skips max-subtraction (|scores| < ~2 by construction); causality = 0/1
mask multiply after exp, only on diagonal-crossing tiles.  The whole layer
is chunk-major (512 tokens) so chunk 0's FFN overlaps chunk 1's
exp-gated attention.  Matmuls bf16, residual stream fp32.  Big weight
matrices stream from DRAM per (chunk, out-tile).
"""

import numpy as np
import ml_dtypes

import concourse.bass as bass
import concourse.mybir as mybir
import concourse.tile as tile
from concourse import bacc
from concourse.bass_utils import run_bass_kernel_spmd

V, E, NH, HD, L, T, B, FF = 50257, 768, 12, 64, 4, 1024, 2, 3072
EPS = 1e-5
P = 128
KE = E // P            # 6 feature subtiles
KF = FF // P           # 24
NT = T // P            # 8 token tiles
NC = 512               # matmul free-dim chunk
NCH = T // NC          # 2 chunks
VP = 12672             # vocab shard per core (99 * 128)
MV = VP // P           # 99
BF16 = mybir.dt.bfloat16
F32 = mybir.dt.float32
AF = mybir.ActivationFunctionType
OP = mybir.AluOpType
BF = ml_dtypes.bfloat16

_CACHE = {}


def _build():
    nc = bacc.Bacc("TRN2", target_bir_lowering=False, debug=False,
                   num_devices=8)

    x0t = nc.declare_dram_parameter("x0t", [E, T], F32, isOutput=False)
    wqk = nc.declare_dram_parameter("wqk", [L, E, 2 * E], BF16, isOutput=False)
    wv = nc.declare_dram_parameter("wv", [L, E, E], BF16, isOutput=False)
    wout = nc.declare_dram_parameter("wout", [L, E, E], BF16, isOutput=False)
    wfc1 = nc.declare_dram_parameter("wfc1", [L, E, FF], BF16, isOutput=False)
    bfc1 = nc.declare_dram_parameter("bfc1", [L, P, KF], F32, isOutput=False)
    wfc2 = nc.declare_dram_parameter("wfc2", [L, FF, E], BF16, isOutput=False)
    bfc2 = nc.declare_dram_parameter("bfc2", [L, P, KE], F32, isOutput=False)
    wemb = nc.declare_dram_parameter("wemb", [E, VP], BF16, isOutput=False)
    maskp = nc.declare_dram_parameter("mask", [P, P], BF16, isOutput=False)
    identp = nc.declare_dram_parameter("ident", [P, P], BF16, isOutput=False)
    out = nc.declare_dram_parameter("out", [VP, T], F32, isOutput=True)

    with tile.TileContext(nc) as tc:
        with (
            tc.tile_pool(name="resident", bufs=1) as res,
            tc.tile_pool(name="wts", bufs=1) as wpool,
            tc.tile_pool(name="acts", bufs=1) as apool,
            tc.tile_pool(name="wstream", bufs=8) as wst,
            tc.tile_pool(name="wstream24", bufs=3) as wst24,
            tc.tile_pool(name="small", bufs=3) as spool,
            tc.tile_pool(name="small2", bufs=2) as spool2,
            tc.tile_pool(name="probs", bufs=2) as ptpool,
            tc.tile_pool(name="ps", bufs=3, space="PSUM") as psp,
            tc.tile_pool(name="ps2", bufs=2, space="PSUM") as psp2,
            tc.tile_pool(name="ps3", bufs=3, space="PSUM") as psp3,
        ):
            # --- resident tiles ---
            x = res.tile([P, KE, T], F32)          # residual stream (xT)
            xhat = res.tile([P, KE, T], BF16)      # normalized, bf16
            mask = res.tile([P, P], BF16)          # diagonal 0/1 block
            ident = res.tile([P, P], BF16)         # PE transpose identity
            v_s = res.tile([P, NT, NH, HD + 1], BF16)  # V + ones column
            ones_c = res.tile([P, 1], BF16)
            negmb = res.tile([P, T], F32)          # -mean broadcast
            rstdb = res.tile([P, T], F32)          # rstd broadcast
            stat = res.tile([1, 2, T], F32)        # negmean / rstd rows
            eps_c = res.tile([1, 1], F32)

            nc.any.memset(ones_c[:], 1.0)
            nc.any.memset(eps_c[:], EPS)
            nc.any.memset(v_s[:, :, :, HD:HD + 1], 1.0)
            nc.sync.dma_start(mask[:], maskp.ap())
            nc.sync.dma_start(ident[:], identp.ap())
            nc.sync.dma_start(x[:], x0t.ap().rearrange("(ko p) t -> p ko t",
                                                       p=P))

            def ln_chunk(c):
                """x chunk c (f32) -> xhat chunk c (bf16), scales folded."""
                cs = slice(c * NC, (c + 1) * NC)
                st_s = psp2.tile([1, NC], F32, tag="av")
                st_q = psp2.tile([1, NC], F32, tag="av")
                xbts = []
                for k in range(KE):
                    xbt = spool.tile([P, NC], BF16, tag="xbt")
                    nc.vector.tensor_copy(out=xbt[:], in_=x[:, k, cs])
                    nc.tensor.matmul(st_s, ones_c[:], xbt[:],
                                     start=(k == 0), stop=(k == KE - 1))
                    xbts.append(xbt)
                for k in range(KE):
                    xsq = spool.tile([P, NC], BF16, tag="xsq")
                    nc.vector.tensor_tensor(
                        xsq[:], xbts[k][:], xbts[k][:], OP.mult)
                    nc.tensor.matmul(st_q, ones_c[:], xsq[:],
                                     start=(k == 0), stop=(k == KE - 1))
                # negmean row (SBUF, fp32); var = sumsq/E - mean^2
                nc.vector.tensor_scalar_mul(stat[:, 0, cs], st_s,
                                            -1.0 / E)
                sq = spool2.tile([1, NC], F32, tag="t_sq")
                nc.vector.tensor_tensor(sq, stat[:, 0, cs], stat[:, 0, cs],
                                        OP.mult)
                u = spool2.tile([1, NC], F32, tag="t_u")
                nc.vector.scalar_tensor_tensor(
                    u, st_q, 1.0 / E, sq, OP.mult, OP.subtract)
                # rstd = exp(-0.5*ln(var+eps)): stays in the Exp act table
                nc.scalar.activation(u, u, AF.Ln, bias=eps_c[:])
                nc.scalar.activation(stat[:, 1, cs], u, AF.Exp, scale=-0.5)
                # broadcast along partitions on the idle GPSIMD engine
                nc.gpsimd.partition_broadcast(negmb[:, cs], stat[:, 0, cs])
                nc.gpsimd.partition_broadcast(rstdb[:, cs], stat[:, 1, cs])
                for k in range(KE):
                    tmp = spool2.tile([P, NC], F32, tag="lntmp")
                    nc.vector.tensor_tensor(
                        tmp, x[:, k, cs], negmb[:, cs], OP.add)
                    nc.vector.tensor_tensor(
                        xhat[:, k, cs], tmp, rstdb[:, cs], OP.mult)

            def w6(dram_ap, m):
                """Stream a (128, KE, 128) lhsT block for output tile m."""
                wt = wst.tile([P, KE, P], BF16, tag="wm6")
                nc.sync.dma_start(
                    wt[:], dram_ap[:, m * P:(m + 1) * P].rearrange(
                        "(ko p) f -> p ko f", p=P))
                return wt

            for l in range(L):
                wv_s = wpool.tile([P, KE, E], BF16, tag="wv")
                b1_s = wpool.tile([P, KF], F32, tag="b1")
                b2_s = wpool.tile([P, KE], F32, tag="b2")
                nc.sync.dma_start(
                    wv_s[:], wv.ap()[l].rearrange("(ko p) f -> p ko f", p=P))
                nc.sync.dma_start(b1_s[:], bfc1.ap()[l])
                nc.sync.dma_start(b2_s[:], bfc2.ap()[l])

                # ---- LN1 + QKV + V, chunk-major ----
                qk_t = apool.tile([P, 2 * KE, T], BF16, tag="qkt")
                for c in range(NCH):
                    cs = slice(c * NC, (c + 1) * NC)
                    ln_chunk(c)
                    for m in range(2 * KE):
                        wt = w6(wqk.ap()[l], m)
                        ps = psp.tile([P, NC], F32, tag="mm")
                        for k in range(KE):
                            nc.tensor.matmul(
                                ps, wt[:, k, :], xhat[:, k, cs],
                                start=(k == 0), stop=(k == KE - 1))
                        nc.vector.tensor_copy(out=qk_t[:, m, cs], in_=ps)
                    for t in range(4 * c, 4 * c + 4):
                        for (f0, fn) in ((0, NC), (NC, E - NC)):
                            ps = psp.tile([P, NC], F32, tag="mm")
                            for k in range(KE):
                                nc.tensor.matmul(
                                    ps[:, :fn], xhat[:, k, t * P:(t + 1) * P],
                                    wv_s[:, k, f0:f0 + fn],
                                    start=(k == 0), stop=(k == KE - 1))
                            nc.vector.tensor_copy(
                                out=v_s[:, t, f0 // HD:(f0 + fn) // HD, 0:HD],
                                in_=ps[:, :fn])

                # ---- attention + out-proj + LN2 + FFN, chunk-major ----
                o_t = apool.tile([P, KE, T], BF16, tag="ot")
                for c in range(NCH):
                    cs = slice(c * NC, (c + 1) * NC)
                    ntk = 4 * (c + 1)   # causal: keep tk tiles 0..ntk-1
                    for h in range(NH):
                        mt, mo = divmod(h * HD, P)
                        q_sl = qk_t[mo:mo + HD, mt, :]
                        k_sl = qk_t[mo:mo + HD, KE + mt, :]
                        ptc = ptpool.tile([P, 8, NC], BF16, tag="ptc")
                        for tk in range(ntk):
                            d = tk - 4 * c
                            d0 = max(d, 0) * P   # cols < d0 are fully masked
                            ps_s = psp3.tile([P, NC], F32, tag="sc")
                            nc.tensor.matmul(
                                ps_s[:, d0:], k_sl[:, tk * P:(tk + 1) * P],
                                q_sl[:, c * NC + d0:(c + 1) * NC],
                                start=True, stop=True)
                            nc.scalar.activation(ptc[:, tk, d0:],
                                                 ps_s[:, d0:], AF.Exp)
                            if d >= 0:   # diagonal block: triangular mask
                                nc.vector.tensor_tensor(
                                    ptc[:, tk, d0:d0 + P],
                                    ptc[:, tk, d0:d0 + P],
                                    mask[:], OP.mult)
                        # A@V transposed: out = probs^T @ [V | 1], so the
                        # softmax denominator lands as column HD.
                        ps_av = psp2.tile([P, 4, P], F32, tag="av")
                        for tq in range(4):
                            nq = 4 * c + tq + 1
                            qs = slice(tq * P, (tq + 1) * P)
                            for i in range(nq):
                                nc.tensor.matmul(
                                    ps_av[:, tq, 0:HD + 1],
                                    ptc[:, i, qs], v_s[:, i, h, :],
                                    start=(i == 0), stop=(i == nq - 1))
                        rc = spool.tile([P, 4], F32, tag="rc")
                        nc.vector.reciprocal_approx_fast(
                            rc, ps_av[:, :, HD])
                        on = spool.tile([P, 4, HD], BF16, tag="on")
                        for tq in range(4):
                            nc.vector.tensor_scalar_mul(
                                on[:, tq, :], ps_av[:, tq, 0:HD],
                                rc[:, tq:tq + 1])
                        tr = psp2.tile([HD, 4, P], BF16, tag="av")
                        for tq in range(4):
                            nc.tensor.transpose(
                                tr[:, tq, :], on[:, tq, :], ident[:])
                        nc.vector.tensor_copy(
                            out=o_t[mo:mo + HD, mt, cs], in_=tr[:, :, :])

                    # ---- output projection + residual, this chunk ----
                    for m in range(KE):
                        wt = w6(wout.ap()[l], m)
                        ps = psp.tile([P, NC], F32, tag="mm")
                        for k in range(KE):
                            nc.tensor.matmul(
                                ps, wt[:, k, :], o_t[:, k, cs],
                                start=(k == 0), stop=(k == KE - 1))
                        nc.vector.tensor_tensor(
                            x[:, m, cs], ps, x[:, m, cs], OP.add)

                    ln_chunk(c)

                    # ---- FFN, this chunk ----
                    h1c = apool.tile([P, KF, NC], BF16, tag="h1c")
                    for m in range(KF):
                        wt = w6(wfc1.ap()[l], m)
                        ps = psp.tile([P, NC], F32, tag="mm")
                        for k in range(KE):
                            nc.tensor.matmul(
                                ps, wt[:, k, :], xhat[:, k, cs],
                                start=(k == 0), stop=(k == KE - 1))
                        nc.scalar.activation(
                            h1c[:, m, :], ps, AF.Gelu, bias=b1_s[:, m:m + 1])
                    for m in range(KE):
                        wt24 = wst24.tile([P, KF, P], BF16, tag="wm24")
                        nc.sync.dma_start(
                            wt24[:],
                            wfc2.ap()[l][:, m * P:(m + 1) * P].rearrange(
                                "(ko p) f -> p ko f", p=P))
                        ps = psp.tile([P, NC], F32, tag="mm")
                        for k in range(KF):
                            nc.tensor.matmul(
                                ps, wt24[:, k, :], h1c[:, k, :],
                                start=(k == 0), stop=(k == KF - 1))
                        nc.vector.affine_then_add(
                            x[:, m, cs], ps, x[:, m, cs],
                            scale=1.0, bias=b2_s[:, m:m + 1])

            # ---- final LN + LM head ----
            for c in range(NCH):
                ln_chunk(c)
            for m in range(MV):
                we_m = w6(wemb.ap(), m)
                for c in range(NCH):
                    cs = slice(c * NC, (c + 1) * NC)
                    ps = psp.tile([P, NC], F32, tag="mm")
                    for k in range(KE):
                        nc.tensor.matmul(
                            ps, we_m[:, k, :], xhat[:, k, cs],
                            start=(k == 0), stop=(k == KE - 1))
                    ot = spool2.tile([P, NC], F32, tag="outsb")
                    nc.vector.tensor_copy(out=ot, in_=ps)
                    nc.sync.dma_start(out.ap()[m * P:(m + 1) * P, cs], ot)

    nc.compile()
    return nc


def _prep(inputs):
    """Host-side: fold LN scales into weights, build per-core input maps."""
    ids = np.asarray(inputs["input_ids"]).astype(np.int64)
    tok = np.asarray(inputs["tok_emb"], np.float32)
    pos = np.asarray(inputs["pos_emb"], np.float32)
    qkv = np.asarray(inputs["qkv_w"], np.float32)
    ow = np.asarray(inputs["out_w"], np.float32)
    f1 = np.asarray(inputs["fc1_w"], np.float32)
    b1 = np.asarray(inputs["fc1_b"], np.float32)
    f2 = np.asarray(inputs["fc2_w"], np.float32)
    b2 = np.asarray(inputs["fc2_b"], np.float32)
    s1 = np.asarray(inputs["ln1_scale"], np.float32)
    bb1 = np.asarray(inputs["ln1_bias"], np.float32)
    s2 = np.asarray(inputs["ln2_scale"], np.float32)
    bb2 = np.asarray(inputs["ln2_bias"], np.float32)
    sf = np.asarray(inputs["lnf_scale"], np.float32)
    bf_ = np.asarray(inputs["lnf_bias"], np.float32)
    # LN biases must be zero for the fold used here (true for this model).
    assert abs(bb1).max() == 0 and abs(bb2).max() == 0 and abs(bf_).max() == 0

    x0 = tok[ids] + pos[None, :, :]                      # (B, T, E)
    x0t = np.ascontiguousarray(x0.transpose(0, 2, 1))    # (B, E, T)

    scale = HD ** -0.5
    wqk_h = np.empty((L, E, 2 * E), BF)
    wv_h = np.empty((L, E, E), BF)
    wo_h = np.empty((L, E, E), BF)
    w1_h = np.empty((L, E, FF), BF)
    w2_h = np.empty((L, FF, E), BF)
    b1_h = np.zeros((L, P, KF), np.float32)
    b2_h = np.zeros((L, P, KE), np.float32)
    for l in range(L):
        wq = (qkv[l, :E] * s1[l][None, :]).T * scale
        wk = (qkv[l, E:2 * E] * s1[l][None, :]).T
        wv_ = (qkv[l, 2 * E:] * s1[l][None, :]).T
        wqk_h[l] = np.concatenate([wq, wk], axis=1).astype(BF)
        wv_h[l] = wv_.astype(BF)
        wo_h[l] = ow[l].T.astype(BF)
        w1_h[l] = (f1[l] * s2[l][None, :]).T.astype(BF)
        w2_h[l] = f2[l].T.astype(BF)
        b1_h[l] = b1[l].reshape(KF, P).T
        b2_h[l] = b2[l].reshape(KE, P).T

    tokp = np.zeros((4 * VP, E), np.float32)
    tokp[:V] = tok * sf[None, :]
    embt = [np.ascontiguousarray(tokp[j * VP:(j + 1) * VP].T).astype(BF)
            for j in range(4)]

    # lower-triangular (inclusive) 0/1 block for the diagonal tiles
    mask_h = (np.arange(P)[:, None] <= np.arange(P)[None, :]).astype(BF)
    ident_h = np.eye(P, dtype=BF)

    in_maps = []
    for c in range(8):
        g, j = c // 4, c % 4
        in_maps.append({
            "x0t": np.ascontiguousarray(x0t[g]),
            "wqk": wqk_h, "wv": wv_h, "wout": wo_h,
            "wfc1": w1_h, "bfc1": b1_h, "wfc2": w2_h, "bfc2": b2_h,
            "wemb": embt[j], "mask": mask_h, "ident": ident_h,
        })
    return in_maps


def kernel(**inputs) -> np.ndarray:
    if "nc" not in _CACHE:
        _CACHE["nc"] = _build()
    nc = _CACHE["nc"]
    in_maps = _prep(inputs)
    res = run_bass_kernel_spmd(nc, in_maps, list(range(8)),
                               **_CACHE.get("run_kwargs", {}))
    _CACHE["last"] = res
    logits = np.empty((B, T, V), np.float32)
    for c in range(8):
        g, j = c // 4, c % 4
        lo = j * VP
        hi = min(V, lo + VP)
        logits[g, :, lo:hi] = res.results[c]["out"][:hi - lo].T
    return logits


# revision 11
# speedup vs baseline: 1.4975x; 1.0053x over previous
"""Distributed Trainium2 Bass kernel for a 4-layer GPT-style transformer.

Sharding: 8 cores = 2 batch groups x 4 vocab shards.
  - core c: batch element g = c//4, vocab shard j = c%4 (12672 ids, padded).
  - Transformer body computed per batch element (replicated within each
    group of 4); tied LM head sharded over vocab.  No collectives.

On-chip layout: activations transposed (features on partitions, tokens on
free).  LayerNorm stats via ones-matmul partition reductions, mean/rstd
broadcast on the (otherwise idle) GPSIMD engine; attention via transposed
scores (k @ q^T), then a second transposition in A@V: probs are the
stationary operand so the A@V output lands queries-on-partitions, with a
ones-column in V producing softmax denominators as a per-partition column.
Normalization is then a cheap per-partition scale; a PE transpose puts
heads back features-on-partitions for the output projection.  Softmax
skips max-subtraction (|scores| < ~2 by construction); causality = 0/1
mask multiply after exp, only on diagonal-crossing tiles.  The whole layer
is chunk-major (512 tokens) so chunk 0's FFN overlaps chunk 1's
exp-gated attention.  Matmuls bf16, residual stream fp32.  Big weight
matrices stream from DRAM per (chunk, out-tile).
"""

import numpy as np
import ml_dtypes

import concourse.bass as bass
import concourse.mybir as mybir
import concourse.tile as tile
from concourse import bacc
from concourse.bass_utils import run_bass_kernel_spmd

V, E, NH, HD, L, T, B, FF = 50257, 768, 12, 64, 4, 1024, 2, 3072
EPS = 1e-5
P = 128
KE = E // P            # 6 feature subtiles
KF = FF // P           # 24
NT = T // P            # 8 token tiles
NC = 512               # matmul free-dim chunk
NCH = T // NC          # 2 chunks
VP = 12672             # vocab shard per core (99 * 128)
MV = VP // P           # 99
BF16 = mybir.dt.bfloat16
F32 = mybir.dt.float32
AF = mybir.ActivationFunctionType
OP = mybir.AluOpType
BF = ml_dtypes.bfloat16

_CACHE = {}


def _build():
    nc = bacc.Bacc("TRN2", target_bir_lowering=False, debug=False,
                   num_devices=8)

    x0t = nc.declare_dram_parameter("x0t", [P, KE, T], F32, isOutput=False)
    wqk = nc.declare_dram_parameter("wqk", [L, 2 * KE, P, KE, P], BF16, isOutput=False)
    wv = nc.declare_dram_parameter("wv", [L, P, KE, E], BF16, isOutput=False)
    wout = nc.declare_dram_parameter("wout", [L, KE, P, KE, P], BF16, isOutput=False)
    wfc1 = nc.declare_dram_parameter("wfc1", [L, KF, P, KE, P], BF16, isOutput=False)
    bfc1 = nc.declare_dram_parameter("bfc1", [L, P, KF], F32, isOutput=False)
    wfc2 = nc.declare_dram_parameter("wfc2", [L, KE, P, KF, P], BF16, isOutput=False)
    bfc2 = nc.declare_dram_parameter("bfc2", [L, P, KE], F32, isOutput=False)
    wemb = nc.declare_dram_parameter("wemb", [MV, P, KE, P], BF16, isOutput=False)
    maskp = nc.declare_dram_parameter("mask", [P, P], BF16, isOutput=False)
    identp = nc.declare_dram_parameter("ident", [P, P], BF16, isOutput=False)
    out = nc.declare_dram_parameter("out", [VP, T], F32, isOutput=True)

    with tile.TileContext(nc) as tc:
        with (
            tc.tile_pool(name="resident", bufs=1) as res,
            tc.tile_pool(name="wts", bufs=1) as wpool,
            tc.tile_pool(name="acts", bufs=1) as apool,
            tc.tile_pool(name="wstream", bufs=8) as wst,
            tc.tile_pool(name="wstream24", bufs=3) as wst24,
            tc.tile_pool(name="small", bufs=3) as spool,
            tc.tile_pool(name="small2", bufs=2) as spool2,
            tc.tile_pool(name="probs", bufs=2) as ptpool,
            tc.tile_pool(name="ps", bufs=3, space="PSUM") as psp,
            tc.tile_pool(name="ps2", bufs=2, space="PSUM") as psp2,
            tc.tile_pool(name="ps3", bufs=3, space="PSUM") as psp3,
        ):
            # --- resident tiles ---
            x = res.tile([P, KE, T], F32)          # residual stream (xT)
            xhat = res.tile([P, KE, T], BF16)      # normalized, bf16
            mask = res.tile([P, P], BF16)          # diagonal 0/1 block
            ident = res.tile([P, P], BF16)         # PE transpose identity
            v_s = res.tile([P, NT, NH, HD + 1], BF16)  # V + ones column
            ones_c = res.tile([P, 1], BF16)
            negmb = res.tile([P, T], F32)          # -mean broadcast
            rstdb = res.tile([P, T], F32)          # rstd broadcast
            stat = res.tile([1, 2, T], F32)        # negmean / rstd rows
            eps_c = res.tile([1, 1], F32)

            nc.any.memset(ones_c[:], 1.0)
            nc.any.memset(eps_c[:], EPS)
            nc.any.memset(v_s[:, :, :, HD:HD + 1], 1.0)
            nc.sync.dma_start(mask[:], maskp.ap())
            nc.sync.dma_start(ident[:], identp.ap())
            nc.sync.dma_start(x[:], x0t.ap())

            def ln_chunk(c):
                """x chunk c (f32) -> xhat chunk c (bf16), scales folded."""
                cs = slice(c * NC, (c + 1) * NC)
                st_s = psp2.tile([1, NC], F32, tag="av")
                st_q = psp2.tile([1, NC], F32, tag="av")
                xbts = []
                for k in range(KE):
                    xbt = spool.tile([P, NC], BF16, tag="xbt")
                    nc.vector.tensor_copy(out=xbt[:], in_=x[:, k, cs])
                    nc.tensor.matmul(st_s, ones_c[:], xbt[:],
                                     start=(k == 0), stop=(k == KE - 1))
                    xbts.append(xbt)
                for k in range(KE):
                    xsq = spool.tile([P, NC], BF16, tag="xsq")
                    nc.vector.tensor_tensor(
                        xsq[:], xbts[k][:], xbts[k][:], OP.mult)
                    nc.tensor.matmul(st_q, ones_c[:], xsq[:],
                                     start=(k == 0), stop=(k == KE - 1))
                # negmean row (SBUF, fp32); var = sumsq/E - mean^2
                nc.vector.tensor_scalar_mul(stat[:, 0, cs], st_s,
                                            -1.0 / E)
                sq = spool2.tile([1, NC], F32, tag="t_sq")
                nc.vector.tensor_tensor(sq, stat[:, 0, cs], stat[:, 0, cs],
                                        OP.mult)
                u = spool2.tile([1, NC], F32, tag="t_u")
                nc.vector.scalar_tensor_tensor(
                    u, st_q, 1.0 / E, sq, OP.mult, OP.subtract)
                # rstd = exp(-0.5*ln(var+eps)): stays in the Exp act table
                nc.scalar.activation(u, u, AF.Ln, bias=eps_c[:])
                nc.scalar.activation(stat[:, 1, cs], u, AF.Exp, scale=-0.5)
                # broadcast along partitions on the idle GPSIMD engine
                nc.gpsimd.partition_broadcast(negmb[:, cs], stat[:, 0, cs])
                nc.gpsimd.partition_broadcast(rstdb[:, cs], stat[:, 1, cs])
                for k in range(KE):
                    tmp = spool2.tile([P, NC], F32, tag="lntmp")
                    nc.vector.tensor_tensor(
                        tmp, x[:, k, cs], negmb[:, cs], OP.add)
                    nc.vector.tensor_tensor(
                        xhat[:, k, cs], tmp, rstdb[:, cs], OP.mult)

            def w6(dram_ap, m):
                """Stream a (128, KE, 128) lhsT block for output tile m."""
                wt = wst.tile([P, KE, P], BF16, tag="wm6")
                nc.sync.dma_start(wt[:], dram_ap[m])
                return wt

            for l in range(L):
                wv_s = wpool.tile([P, KE, E], BF16, tag="wv")
                b1_s = wpool.tile([P, KF], F32, tag="b1")
                b2_s = wpool.tile([P, KE], F32, tag="b2")
                nc.sync.dma_start(wv_s[:], wv.ap()[l])
                nc.sync.dma_start(b1_s[:], bfc1.ap()[l])
                nc.sync.dma_start(b2_s[:], bfc2.ap()[l])

                # ---- LN1 + QKV + V, chunk-major ----
                qk_t = apool.tile([P, 2 * KE, T], BF16, tag="qkt")
                for c in range(NCH):
                    cs = slice(c * NC, (c + 1) * NC)
                    ln_chunk(c)
                    for m in range(2 * KE):
                        wt = w6(wqk.ap()[l], m)
                        ps = psp.tile([P, NC], F32, tag="mm")
                        for k in range(KE):
                            nc.tensor.matmul(
                                ps, wt[:, k, :], xhat[:, k, cs],
                                start=(k == 0), stop=(k == KE - 1))
                        nc.vector.tensor_copy(out=qk_t[:, m, cs], in_=ps)
                    for t in range(4 * c, 4 * c + 4):
                        for (f0, fn) in ((0, NC), (NC, E - NC)):
                            ps = psp.tile([P, NC], F32, tag="mm")
                            for k in range(KE):
                                nc.tensor.matmul(
                                    ps[:, :fn], xhat[:, k, t * P:(t + 1) * P],
                                    wv_s[:, k, f0:f0 + fn],
                                    start=(k == 0), stop=(k == KE - 1))
                            nc.vector.tensor_copy(
                                out=v_s[:, t, f0 // HD:(f0 + fn) // HD, 0:HD],
                                in_=ps[:, :fn])

                # ---- attention + out-proj + LN2 + FFN, chunk-major ----
                o_t = apool.tile([P, KE, T], BF16, tag="ot")
                for c in range(NCH):
                    cs = slice(c * NC, (c + 1) * NC)
                    ntk = 4 * (c + 1)   # causal: keep tk tiles 0..ntk-1
                    for h in range(NH):
                        mt, mo = divmod(h * HD, P)
                        q_sl = qk_t[mo:mo + HD, mt, :]
                        k_sl = qk_t[mo:mo + HD, KE + mt, :]
                        ptc = ptpool.tile([P, 8, NC], BF16, tag="ptc")
                        for tk in range(ntk):
                            d = tk - 4 * c
                            d0 = max(d, 0) * P   # cols < d0 are fully masked
                            ps_s = psp3.tile([P, NC], F32, tag="sc")
                            nc.tensor.matmul(
                                ps_s[:, d0:], k_sl[:, tk * P:(tk + 1) * P],
                                q_sl[:, c * NC + d0:(c + 1) * NC],
                                start=True, stop=True)
                            nc.scalar.activation(ptc[:, tk, d0:],
                                                 ps_s[:, d0:], AF.Exp)
                            if d >= 0:   # diagonal block: triangular mask
                                nc.vector.tensor_tensor(
                                    ptc[:, tk, d0:d0 + P],
                                    ptc[:, tk, d0:d0 + P],
                                    mask[:], OP.mult)
                        # A@V transposed: out = probs^T @ [V | 1], so the
                        # softmax denominator lands as column HD.
                        ps_av = psp2.tile([P, 4, P], F32, tag="av")
                        for tq in range(4):
                            nq = 4 * c + tq + 1
                            qs = slice(tq * P, (tq + 1) * P)
                            for i in range(nq):
                                nc.tensor.matmul(
                                    ps_av[:, tq, 0:HD + 1],
                                    ptc[:, i, qs], v_s[:, i, h, :],
                                    start=(i == 0), stop=(i == nq - 1))
                        rc = spool.tile([P, 4], F32, tag="rc")
                        nc.vector.reciprocal_approx_fast(
                            rc, ps_av[:, :, HD])
                        on = spool.tile([P, 4, HD], BF16, tag="on")
                        for tq in range(4):
                            nc.vector.tensor_scalar_mul(
                                on[:, tq, :], ps_av[:, tq, 0:HD],
                                rc[:, tq:tq + 1])
                        tr = psp2.tile([HD, 4, P], BF16, tag="av")
                        for tq in range(4):
                            nc.tensor.transpose(
                                tr[:, tq, :], on[:, tq, :], ident[:])
                        nc.vector.tensor_copy(
                            out=o_t[mo:mo + HD, mt, cs], in_=tr[:, :, :])

                    # ---- output projection + residual, this chunk ----
                    for m in range(KE):
                        wt = w6(wout.ap()[l], m)
                        ps = psp.tile([P, NC], F32, tag="mm")
                        for k in range(KE):
                            nc.tensor.matmul(
                                ps, wt[:, k, :], o_t[:, k, cs],
                                start=(k == 0), stop=(k == KE - 1))
                        nc.vector.tensor_tensor(
                            x[:, m, cs], ps, x[:, m, cs], OP.add)

                    ln_chunk(c)

                    # ---- FFN, this chunk ----
                    h1c = apool.tile([P, KF, NC], BF16, tag="h1c")
                    for m in range(KF):
                        wt = w6(wfc1.ap()[l], m)
                        ps = psp.tile([P, NC], F32, tag="mm")
                        for k in range(KE):
                            nc.tensor.matmul(
                                ps, wt[:, k, :], xhat[:, k, cs],
                                start=(k == 0), stop=(k == KE - 1))
                        nc.scalar.activation(
                            h1c[:, m, :], ps, AF.Gelu, bias=b1_s[:, m:m + 1])
                    for m in range(KE):
                        wt24 = wst24.tile([P, KF, P], BF16, tag="wm24")
                        nc.sync.dma_start(wt24[:], wfc2.ap()[l][m])
                        ps = psp.tile([P, NC], F32, tag="mm")
                        for k in range(KF):
                            nc.tensor.matmul(
                                ps, wt24[:, k, :], h1c[:, k, :],
                                start=(k == 0), stop=(k == KF - 1))
                        nc.vector.affine_then_add(
                            x[:, m, cs], ps, x[:, m, cs],
                            scale=1.0, bias=b2_s[:, m:m + 1])

            # ---- final LN + LM head ----
            for c in range(NCH):
                ln_chunk(c)
            for m in range(MV):
                we_m = w6(wemb.ap(), m)
                for c in range(NCH):
                    cs = slice(c * NC, (c + 1) * NC)
                    ps = psp.tile([P, NC], F32, tag="mm")
                    for k in range(KE):
                        nc.tensor.matmul(
                            ps, we_m[:, k, :], xhat[:, k, cs],
                            start=(k == 0), stop=(k == KE - 1))
                    ot = spool2.tile([P, NC], F32, tag="outsb")
                    nc.vector.tensor_copy(out=ot, in_=ps)
                    nc.sync.dma_start(out.ap()[m * P:(m + 1) * P, cs], ot)

    nc.compile()
    return nc


def _prep(inputs):
    """Host-side: fold LN scales into weights, build per-core input maps."""
    ids = np.asarray(inputs["input_ids"]).astype(np.int64)
    tok = np.asarray(inputs["tok_emb"], np.float32)
    pos = np.asarray(inputs["pos_emb"], np.float32)
    qkv = np.asarray(inputs["qkv_w"], np.float32)
    ow = np.asarray(inputs["out_w"], np.float32)
    f1 = np.asarray(inputs["fc1_w"], np.float32)
    b1 = np.asarray(inputs["fc1_b"], np.float32)
    f2 = np.asarray(inputs["fc2_w"], np.float32)
    b2 = np.asarray(inputs["fc2_b"], np.float32)
    s1 = np.asarray(inputs["ln1_scale"], np.float32)
    bb1 = np.asarray(inputs["ln1_bias"], np.float32)
    s2 = np.asarray(inputs["ln2_scale"], np.float32)
    bb2 = np.asarray(inputs["ln2_bias"], np.float32)
    sf = np.asarray(inputs["lnf_scale"], np.float32)
    bf_ = np.asarray(inputs["lnf_bias"], np.float32)
    # LN biases must be zero for the fold used here (true for this model).
    assert abs(bb1).max() == 0 and abs(bb2).max() == 0 and abs(bf_).max() == 0

    x0 = tok[ids] + pos[None, :, :]                      # (B, T, E)
    x0t = np.ascontiguousarray(
        x0.transpose(0, 2, 1).reshape(B, KE, P, T).transpose(0, 2, 1, 3))

    scale = HD ** -0.5

    def tile6(w):
        # (E, M) -> (M//P, P, KE, P): [m, p, ko, f] = w[ko*P+p, m*P+f]
        Ein, M = w.shape
        return np.ascontiguousarray(
            w.reshape(Ein // P, P, M // P, P).transpose(2, 1, 0, 3))

    wqk_h = np.empty((L, 2 * KE, P, KE, P), BF)
    wv_h = np.empty((L, P, KE, E), BF)
    wo_h = np.empty((L, KE, P, KE, P), BF)
    w1_h = np.empty((L, KF, P, KE, P), BF)
    w2_h = np.empty((L, KE, P, KF, P), BF)
    b1_h = np.zeros((L, P, KF), np.float32)
    b2_h = np.zeros((L, P, KE), np.float32)
    for l in range(L):
        wq = (qkv[l, :E] * s1[l][None, :]).T * scale
        wk = (qkv[l, E:2 * E] * s1[l][None, :]).T
        wv_ = (qkv[l, 2 * E:] * s1[l][None, :]).T
        wqk_h[l] = tile6(np.concatenate([wq, wk], axis=1)).astype(BF)
        wv_h[l] = np.ascontiguousarray(
            wv_.reshape(KE, P, E).transpose(1, 0, 2)).astype(BF)
        wo_h[l] = tile6(ow[l].T).astype(BF)
        w1_h[l] = tile6((f1[l] * s2[l][None, :]).T).astype(BF)
        w2_h[l] = tile6(f2[l].T).astype(BF)
        b1_h[l] = b1[l].reshape(KF, P).T
        b2_h[l] = b2[l].reshape(KE, P).T

    tokp = np.zeros((4 * VP, E), np.float32)
    tokp[:V] = tok * sf[None, :]
    embt = [tile6(np.ascontiguousarray(
                tokp[j * VP:(j + 1) * VP].T).astype(np.float32)).astype(BF)
            for j in range(4)]

    # lower-triangular (inclusive) 0/1 block for the diagonal tiles
    mask_h = (np.arange(P)[:, None] <= np.arange(P)[None, :]).astype(BF)
    ident_h = np.eye(P, dtype=BF)

    in_maps = []
    for c in range(8):
        g, j = c // 4, c % 4
        in_maps.append({
            "x0t": np.ascontiguousarray(x0t[g]),
            "wqk": wqk_h, "wv": wv_h, "wout": wo_h,
            "wfc1": w1_h, "bfc1": b1_h, "wfc2": w2_h, "bfc2": b2_h,
            "wemb": embt[j], "mask": mask_h, "ident": ident_h,
        })
    return in_maps


def kernel(**inputs) -> np.ndarray:
    if "nc" not in _CACHE:
        _CACHE["nc"] = _build()
    nc = _CACHE["nc"]
    in_maps = _prep(inputs)
    res = run_bass_kernel_spmd(nc, in_maps, list(range(8)),
                               **_CACHE.get("run_kwargs", {}))
    _CACHE["last"] = res
    logits = np.empty((B, T, V), np.float32)
    for c in range(8):
        g, j = c // 4, c % 4
        lo = j * VP
        hi = min(V, lo + VP)
        logits[g, :, lo:hi] = res.results[c]["out"][:hi - lo].T
    return logits


# revision 12
# speedup vs baseline: 1.6097x; 1.0749x over previous
"""Distributed Trainium2 Bass kernel for a 4-layer GPT-style transformer.

Sharding: 8 cores = 2 batch groups x 4 vocab shards.
  - core c: batch element g = c//4, vocab shard j = c%4 (12672 ids, padded).
  - Transformer body computed per batch element (replicated within each
    group of 4); tied LM head sharded over vocab.  No collectives.

On-chip layout: activations transposed (features on partitions, tokens on
free).  LayerNorm stats via ones-matmul partition reductions, mean/rstd
broadcast on the (otherwise idle) GPSIMD engine; attention via transposed
scores (k @ q^T), then a second transposition in A@V: probs are the
stationary operand so the A@V output lands queries-on-partitions, with a
ones-column in V producing softmax denominators as a per-partition column.
Normalization is then a cheap per-partition scale; a PE transpose puts
heads back features-on-partitions for the output projection.  Softmax
skips max-subtraction (|scores| < ~2 by construction); causality = 0/1
mask multiply after exp, only on diagonal-crossing tiles.  The whole layer
is chunk-major (512 tokens) so chunk 0's FFN overlaps chunk 1's
exp-gated attention.  Matmuls bf16, residual stream fp32.  Big weight
matrices stream from DRAM per (chunk, out-tile).
"""

import numpy as np
import ml_dtypes

import concourse.bass as bass
import concourse.mybir as mybir
import concourse.tile as tile
from concourse import bacc
from concourse.bass_utils import run_bass_kernel_spmd

V, E, NH, HD, L, T, B, FF = 50257, 768, 12, 64, 4, 1024, 2, 3072
EPS = 1e-5
P = 128
KE = E // P            # 6 feature subtiles
KF = FF // P           # 24
NT = T // P            # 8 token tiles
NC = 512               # matmul free-dim chunk
NCH = T // NC          # 2 chunks
VP = 12672             # vocab shard per core (99 * 128)
MV = VP // P           # 99
BF16 = mybir.dt.bfloat16
F32 = mybir.dt.float32
AF = mybir.ActivationFunctionType
OP = mybir.AluOpType
BF = ml_dtypes.bfloat16

_CACHE = {}


def _build():
    nc = bacc.Bacc("TRN2", target_bir_lowering=False, debug=False,
                   num_devices=8)

    x0t = nc.declare_dram_parameter("x0t", [P, KE, T], F32, isOutput=False)
    wqk = nc.declare_dram_parameter("wqk", [L, 2 * KE, P, KE, P], BF16, isOutput=False)
    wv = nc.declare_dram_parameter("wv", [L, P, KE, E], BF16, isOutput=False)
    wout = nc.declare_dram_parameter("wout", [L, KE, P, KE, P], BF16, isOutput=False)
    wfc1 = nc.declare_dram_parameter("wfc1", [L, KF, P, KE, P], BF16, isOutput=False)
    bfc1 = nc.declare_dram_parameter("bfc1", [L, P, KF], F32, isOutput=False)
    wfc2 = nc.declare_dram_parameter("wfc2", [L, KE, P, KF, P], BF16, isOutput=False)
    bfc2 = nc.declare_dram_parameter("bfc2", [L, P, KE], F32, isOutput=False)
    wemb = nc.declare_dram_parameter("wemb", [MV, P, KE, P], BF16, isOutput=False)
    maskp = nc.declare_dram_parameter("mask", [P, P], BF16, isOutput=False)
    identp = nc.declare_dram_parameter("ident", [P, P], BF16, isOutput=False)
    out = nc.declare_dram_parameter("out", [VP, T], BF16, isOutput=True)

    with tile.TileContext(nc) as tc:
        with (
            tc.tile_pool(name="resident", bufs=1) as res,
            tc.tile_pool(name="wts", bufs=1) as wpool,
            tc.tile_pool(name="acts", bufs=1) as apool,
            tc.tile_pool(name="wstream", bufs=8) as wst,
            tc.tile_pool(name="wstream24", bufs=3) as wst24,
            tc.tile_pool(name="small", bufs=3) as spool,
            tc.tile_pool(name="small2", bufs=2) as spool2,
            tc.tile_pool(name="outp", bufs=6) as opool,
            tc.tile_pool(name="probs", bufs=2) as ptpool,
            tc.tile_pool(name="ps", bufs=3, space="PSUM") as psp,
            tc.tile_pool(name="ps2", bufs=2, space="PSUM") as psp2,
            tc.tile_pool(name="ps3", bufs=3, space="PSUM") as psp3,
        ):
            # --- resident tiles ---
            x = res.tile([P, KE, T], F32)          # residual stream (xT)
            xhat = res.tile([P, KE, T], BF16)      # normalized, bf16
            mask = res.tile([P, P], BF16)          # diagonal 0/1 block
            ident = res.tile([P, P], BF16)         # PE transpose identity
            v_s = res.tile([P, NT, NH, HD + 1], BF16)  # V + ones column
            ones_c = res.tile([P, 1], BF16)
            negmb = res.tile([P, T], F32)          # -mean broadcast
            rstdb = res.tile([P, T], F32)          # rstd broadcast
            stat = res.tile([1, 2, T], F32)        # negmean / rstd rows
            eps_c = res.tile([1, 1], F32)

            nc.any.memset(ones_c[:], 1.0)
            nc.any.memset(eps_c[:], EPS)
            nc.any.memset(v_s[:, :, :, HD:HD + 1], 1.0)
            nc.sync.dma_start(mask[:], maskp.ap())
            nc.sync.dma_start(ident[:], identp.ap())
            nc.sync.dma_start(x[:], x0t.ap())

            def ln_chunk(c):
                """x chunk c (f32) -> xhat chunk c (bf16), scales folded."""
                cs = slice(c * NC, (c + 1) * NC)
                st_s = psp2.tile([1, NC], F32, tag="av")
                st_q = psp2.tile([1, NC], F32, tag="av")
                xbts = []
                for k in range(KE):
                    xbt = spool.tile([P, NC], BF16, tag="xbt")
                    nc.vector.tensor_copy(out=xbt[:], in_=x[:, k, cs])
                    nc.tensor.matmul(st_s, ones_c[:], xbt[:],
                                     start=(k == 0), stop=(k == KE - 1))
                    xbts.append(xbt)
                for k in range(KE):
                    xsq = spool.tile([P, NC], BF16, tag="xsq")
                    nc.vector.tensor_tensor(
                        xsq[:], xbts[k][:], xbts[k][:], OP.mult)
                    nc.tensor.matmul(st_q, ones_c[:], xsq[:],
                                     start=(k == 0), stop=(k == KE - 1))
                # negmean row (SBUF, fp32); var = sumsq/E - mean^2
                nc.vector.tensor_scalar_mul(stat[:, 0, cs], st_s,
                                            -1.0 / E)
                sq = spool2.tile([1, NC], F32, tag="t_sq")
                nc.vector.tensor_tensor(sq, stat[:, 0, cs], stat[:, 0, cs],
                                        OP.mult)
                u = spool2.tile([1, NC], F32, tag="t_u")
                nc.vector.scalar_tensor_tensor(
                    u, st_q, 1.0 / E, sq, OP.mult, OP.subtract)
                # rstd = exp(-0.5*ln(var+eps)): stays in the Exp act table
                nc.scalar.activation(u, u, AF.Ln, bias=eps_c[:])
                nc.scalar.activation(stat[:, 1, cs], u, AF.Exp, scale=-0.5)
                # broadcast along partitions on the idle GPSIMD engine
                nc.gpsimd.partition_broadcast(negmb[:, cs], stat[:, 0, cs])
                nc.gpsimd.partition_broadcast(rstdb[:, cs], stat[:, 1, cs])
                for k in range(KE):
                    tmp = spool2.tile([P, NC], F32, tag="lntmp")
                    nc.vector.tensor_tensor(
                        tmp, x[:, k, cs], negmb[:, cs], OP.add)
                    nc.vector.tensor_tensor(
                        xhat[:, k, cs], tmp, rstdb[:, cs], OP.mult)

            def w6(dram_ap, m):
                """Stream a (128, KE, 128) lhsT block for output tile m."""
                wt = wst.tile([P, KE, P], BF16, tag="wm6")
                nc.sync.dma_start(wt[:], dram_ap[m])
                return wt

            for l in range(L):
                wv_s = wpool.tile([P, KE, E], BF16, tag="wv")
                b1_s = wpool.tile([P, KF], F32, tag="b1")
                b2_s = wpool.tile([P, KE], F32, tag="b2")
                nc.sync.dma_start(wv_s[:], wv.ap()[l])
                nc.sync.dma_start(b1_s[:], bfc1.ap()[l])
                nc.sync.dma_start(b2_s[:], bfc2.ap()[l])

                # ---- LN1 + QKV + V, chunk-major ----
                qk_t = apool.tile([P, 2 * KE, T], BF16, tag="qkt")
                for c in range(NCH):
                    cs = slice(c * NC, (c + 1) * NC)
                    ln_chunk(c)
                    for m in range(2 * KE):
                        wt = w6(wqk.ap()[l], m)
                        ps = psp.tile([P, NC], F32, tag="mm")
                        for k in range(KE):
                            nc.tensor.matmul(
                                ps, wt[:, k, :], xhat[:, k, cs],
                                start=(k == 0), stop=(k == KE - 1))
                        nc.vector.tensor_copy(out=qk_t[:, m, cs], in_=ps)
                    for t in range(4 * c, 4 * c + 4):
                        for (f0, fn) in ((0, NC), (NC, E - NC)):
                            ps = psp.tile([P, NC], F32, tag="mm")
                            for k in range(KE):
                                nc.tensor.matmul(
                                    ps[:, :fn], xhat[:, k, t * P:(t + 1) * P],
                                    wv_s[:, k, f0:f0 + fn],
                                    start=(k == 0), stop=(k == KE - 1))
                            nc.vector.tensor_copy(
                                out=v_s[:, t, f0 // HD:(f0 + fn) // HD, 0:HD],
                                in_=ps[:, :fn])

                # ---- attention + out-proj + LN2 + FFN, chunk-major ----
                o_t = apool.tile([P, KE, T], BF16, tag="ot")
                for c in range(NCH):
                    cs = slice(c * NC, (c + 1) * NC)
                    ntk = 4 * (c + 1)   # causal: keep tk tiles 0..ntk-1
                    for h in range(NH):
                        mt, mo = divmod(h * HD, P)
                        q_sl = qk_t[mo:mo + HD, mt, :]
                        k_sl = qk_t[mo:mo + HD, KE + mt, :]
                        ptc = ptpool.tile([P, 8, NC], BF16, tag="ptc")
                        for tk in range(ntk):
                            d = tk - 4 * c
                            d0 = max(d, 0) * P   # cols < d0 are fully masked
                            ps_s = psp3.tile([P, NC], F32, tag="sc")
                            nc.tensor.matmul(
                                ps_s[:, d0:], k_sl[:, tk * P:(tk + 1) * P],
                                q_sl[:, c * NC + d0:(c + 1) * NC],
                                start=True, stop=True)
                            nc.scalar.activation(ptc[:, tk, d0:],
                                                 ps_s[:, d0:], AF.Exp)
                            if d >= 0:   # diagonal block: triangular mask
                                nc.vector.tensor_tensor(
                                    ptc[:, tk, d0:d0 + P],
                                    ptc[:, tk, d0:d0 + P],
                                    mask[:], OP.mult)
                        # A@V transposed: out = probs^T @ [V | 1], so the
                        # softmax denominator lands as column HD.
                        ps_av = psp2.tile([P, 4, P], F32, tag="av")
                        for tq in range(4):
                            nq = 4 * c + tq + 1
                            qs = slice(tq * P, (tq + 1) * P)
                            for i in range(nq):
                                nc.tensor.matmul(
                                    ps_av[:, tq, 0:HD + 1],
                                    ptc[:, i, qs], v_s[:, i, h, :],
                                    start=(i == 0), stop=(i == nq - 1))
                        rc = spool.tile([P, 4], F32, tag="rc")
                        nc.vector.reciprocal_approx_fast(
                            rc, ps_av[:, :, HD])
                        on = spool.tile([P, 4, HD], BF16, tag="on")
                        for tq in range(4):
                            nc.vector.tensor_scalar_mul(
                                on[:, tq, :], ps_av[:, tq, 0:HD],
                                rc[:, tq:tq + 1])
                        tr = psp2.tile([HD, 4, P], BF16, tag="av")
                        for tq in range(4):
                            nc.tensor.transpose(
                                tr[:, tq, :], on[:, tq, :], ident[:])
                        nc.vector.tensor_copy(
                            out=o_t[mo:mo + HD, mt, cs], in_=tr[:, :, :])

                    # ---- output projection + residual, this chunk ----
                    for m in range(KE):
                        wt = w6(wout.ap()[l], m)
                        ps = psp.tile([P, NC], F32, tag="mm")
                        for k in range(KE):
                            nc.tensor.matmul(
                                ps, wt[:, k, :], o_t[:, k, cs],
                                start=(k == 0), stop=(k == KE - 1))
                        nc.vector.tensor_tensor(
                            x[:, m, cs], ps, x[:, m, cs], OP.add)

                    ln_chunk(c)

                    # ---- FFN, this chunk ----
                    h1c = apool.tile([P, KF, NC], BF16, tag="h1c")
                    for m in range(KF):
                        wt = w6(wfc1.ap()[l], m)
                        ps = psp.tile([P, NC], F32, tag="mm")
                        for k in range(KE):
                            nc.tensor.matmul(
                                ps, wt[:, k, :], xhat[:, k, cs],
                                start=(k == 0), stop=(k == KE - 1))
                        nc.scalar.activation(
                            h1c[:, m, :], ps, AF.Gelu, bias=b1_s[:, m:m + 1])
                    for m in range(KE):
                        wt24 = wst24.tile([P, KF, P], BF16, tag="wm24")
                        nc.sync.dma_start(wt24[:], wfc2.ap()[l][m])
                        ps = psp.tile([P, NC], F32, tag="mm")
                        for k in range(KF):
                            nc.tensor.matmul(
                                ps, wt24[:, k, :], h1c[:, k, :],
                                start=(k == 0), stop=(k == KF - 1))
                        nc.vector.affine_then_add(
                            x[:, m, cs], ps, x[:, m, cs],
                            scale=1.0, bias=b2_s[:, m:m + 1])

            # ---- final LN + LM head ----
            for c in range(NCH):
                ln_chunk(c)
            for m in range(MV):
                we_m = w6(wemb.ap(), m)
                for c in range(NCH):
                    cs = slice(c * NC, (c + 1) * NC)
                    ps = psp.tile([P, NC], F32, tag="mm")
                    for k in range(KE):
                        nc.tensor.matmul(
                            ps, we_m[:, k, :], xhat[:, k, cs],
                            start=(k == 0), stop=(k == KE - 1))
                    ot = opool.tile([P, NC], BF16, tag="outsb")
                    nc.vector.tensor_copy(out=ot, in_=ps)
                    nc.sync.dma_start(out.ap()[m * P:(m + 1) * P, cs], ot)

    nc.compile()
    return nc


def _prep(inputs):
    """Host-side: fold LN scales into weights, build per-core input maps."""
    ids = np.asarray(inputs["input_ids"]).astype(np.int64)
    tok = np.asarray(inputs["tok_emb"], np.float32)
    pos = np.asarray(inputs["pos_emb"], np.float32)
    qkv = np.asarray(inputs["qkv_w"], np.float32)
    ow = np.asarray(inputs["out_w"], np.float32)
    f1 = np.asarray(inputs["fc1_w"], np.float32)
    b1 = np.asarray(inputs["fc1_b"], np.float32)
    f2 = np.asarray(inputs["fc2_w"], np.float32)
    b2 = np.asarray(inputs["fc2_b"], np.float32)
    s1 = np.asarray(inputs["ln1_scale"], np.float32)
    bb1 = np.asarray(inputs["ln1_bias"], np.float32)
    s2 = np.asarray(inputs["ln2_scale"], np.float32)
    bb2 = np.asarray(inputs["ln2_bias"], np.float32)
    sf = np.asarray(inputs["lnf_scale"], np.float32)
    bf_ = np.asarray(inputs["lnf_bias"], np.float32)
    # LN biases must be zero for the fold used here (true for this model).
    assert abs(bb1).max() == 0 and abs(bb2).max() == 0 and abs(bf_).max() == 0

    x0 = tok[ids] + pos[None, :, :]                      # (B, T, E)
    x0t = np.ascontiguousarray(
        x0.transpose(0, 2, 1).reshape(B, KE, P, T).transpose(0, 2, 1, 3))

    scale = HD ** -0.5

    def tile6(w):
        # (E, M) -> (M//P, P, KE, P): [m, p, ko, f] = w[ko*P+p, m*P+f]
        Ein, M = w.shape
        return np.ascontiguousarray(
            w.reshape(Ein // P, P, M // P, P).transpose(2, 1, 0, 3))

    wqk_h = np.empty((L, 2 * KE, P, KE, P), BF)
    wv_h = np.empty((L, P, KE, E), BF)
    wo_h = np.empty((L, KE, P, KE, P), BF)
    w1_h = np.empty((L, KF, P, KE, P), BF)
    w2_h = np.empty((L, KE, P, KF, P), BF)
    b1_h = np.zeros((L, P, KF), np.float32)
    b2_h = np.zeros((L, P, KE), np.float32)
    for l in range(L):
        wq = (qkv[l, :E] * s1[l][None, :]).T * scale
        wk = (qkv[l, E:2 * E] * s1[l][None, :]).T
        wv_ = (qkv[l, 2 * E:] * s1[l][None, :]).T
        wqk_h[l] = tile6(np.concatenate([wq, wk], axis=1)).astype(BF)
        wv_h[l] = np.ascontiguousarray(
            wv_.reshape(KE, P, E).transpose(1, 0, 2)).astype(BF)
        wo_h[l] = tile6(ow[l].T).astype(BF)
        w1_h[l] = tile6((f1[l] * s2[l][None, :]).T).astype(BF)
        w2_h[l] = tile6(f2[l].T).astype(BF)
        b1_h[l] = b1[l].reshape(KF, P).T
        b2_h[l] = b2[l].reshape(KE, P).T

    tokp = np.zeros((4 * VP, E), np.float32)
    tokp[:V] = tok * sf[None, :]
    embt = [tile6(np.ascontiguousarray(
                tokp[j * VP:(j + 1) * VP].T).astype(np.float32)).astype(BF)
            for j in range(4)]

    # lower-triangular (inclusive) 0/1 block for the diagonal tiles
    mask_h = (np.arange(P)[:, None] <= np.arange(P)[None, :]).astype(BF)
    ident_h = np.eye(P, dtype=BF)

    in_maps = []
    for c in range(8):
        g, j = c // 4, c % 4
        in_maps.append({
            "x0t": np.ascontiguousarray(x0t[g]),
            "wqk": wqk_h, "wv": wv_h, "wout": wo_h,
            "wfc1": w1_h, "bfc1": b1_h, "wfc2": w2_h, "bfc2": b2_h,
            "wemb": embt[j], "mask": mask_h, "ident": ident_h,
        })
    return in_maps


def kernel(**inputs) -> np.ndarray:
    if "nc" not in _CACHE:
        _CACHE["nc"] = _build()
    nc = _CACHE["nc"]
    in_maps = _prep(inputs)
    res = run_bass_kernel_spmd(nc, in_maps, list(range(8)),
                               **_CACHE.get("run_kwargs", {}))
    _CACHE["last"] = res
    logits = np.empty((B, T, V), np.float32)
    for c in range(8):
        g, j = c // 4, c % 4
        lo = j * VP
        hi = min(V, lo + VP)
        logits[g, :, lo:hi] = res.results[c]["out"][:hi - lo].T.astype(
            np.float32)
    return logits
